# revision 8
# baseline (speedup 1.0000x reference)
"""3-branch GCN (DGL GraphConv x3 + max-pool + MLP head) on 8 TRN2 NeuronCores.

Sharding: destination nodes (2500/core). Per layer, each core gathers src rows
from a replicated DRAM table (per-chunk indirect DMA), aggregates via one-hot
fp16 matmuls into PSUM, applies the dense W matmul per dst tile, and the layer
output shards are AllGathered for the next layer. Max-pool is local + a final
AllReduce(max); the tiny MLP head runs replicated on every core.
"""
import os
import numpy as np
import concourse.bass as bass
import concourse.bacc as bacc
import concourse.tile as tile
import concourse.mybir as mybir
from concourse.bass_utils import run_bass_kernel_spmd

NC_ = 8
N = 20000
E = 320000
SH = N // NC_          # 2500 nodes per core
NT = 20                # dst tiles per core (19 full + 68-node partial)
D_IN, D_H = 128, 304
DPAD = 384             # fp16 row pad for 768B (256B-mult) rows
f16, f32 = mybir.dt.float16, mybir.dt.float32
i32 = mybir.dt.int32
AF = mybir.ActivationFunctionType
core_ids = list(range(NC_))


def _prep_graph(src, dst):
    """Per-core chunked edge metadata with core-uniform chunk counts."""
    src = np.asarray(src).astype(np.int64)
    dst = np.asarray(dst).astype(np.int64)
    outdeg = np.bincount(src, minlength=N).clip(1).astype(np.float32)
    indeg = np.bincount(dst, minlength=N).clip(1).astype(np.float32)
    per_core = []
    for c in range(NC_):
        m = (dst // SH) == c
        es, ed = src[m], dst[m] - c * SH
        tiles = []
        for t in range(NT):
            tm = (ed // 128) == t
            tiles.append((es[tm], ed[tm] - t * 128))
        per_core.append(tiles)
    # uniform chunk count per tile slot
    Ck = [max(int(np.ceil(len(per_core[c][t][0]) / 128)) for c in range(NC_)) or 1
          for t in range(NT)]
    nchunks = sum(Ck)
    offs = np.full((NC_, 128, nchunks), 0, np.int32)      # pad -> row 0 (S row zero)
    drel = np.full((NC_, 128, nchunks), -1.0, np.float32)  # -1 -> zero S row
    for c in range(NC_):
        j0 = 0
        for t in range(NT):
            es, er = per_core[c][t]
            npad = Ck[t] * 128
            e_s = np.full(npad, 0, np.int64)
            e_r = np.full(npad, -1.0, np.float32)
            e_s[:len(es)] = es
            e_r[:len(er)] = er
            offs[c, :, j0:j0 + Ck[t]] = e_s.reshape(Ck[t], 128).T
            drel[c, :, j0:j0 + Ck[t]] = e_r.reshape(Ck[t], 128).T
            j0 += Ck[t]
    # per-core slot-ordered degree arrays [128, NT]
    ind = np.ones((NC_, 128, NT), np.float32)
    outd = np.ones((NC_, 128, NT), np.float32)
    for c in range(NC_):
        for t in range(NT):
            lo = c * SH + t * 128
            hi = min(lo + 128, (c + 1) * SH)
            ind[c, :hi - lo, t] = indeg[lo:hi]
            outd[c, :hi - lo, t] = outdeg[lo:hi]
    # full outdeg [128, 157] node n -> [n%128, n//128] (lane-major tiles)
    odf = np.ones((128, 157), np.float32)
    odf.reshape(-1)[:0] = 0
    tmp = np.ones(157 * 128, np.float32)
    tmp[:N] = outdeg
    odf = tmp.reshape(157, 128).T.copy()
    return Ck, offs, drel, ind, outd, odf


def _build(g_meta):
    nc = bacc.Bacc(None, target_bir_lowering=False)
    ext = {}
    for g in range(3):
        Ck, offs, drel, ind, outd, odf = g_meta[g]
        nch = sum(Ck)
        ext[f"x{g}"] = nc.dram_tensor(f"x{g}", [N, D_IN], f32, kind="ExternalInput")
        ext[f"off{g}"] = nc.dram_tensor(f"off{g}", [128, nch], i32, kind="ExternalInput")
        ext[f"dr{g}"] = nc.dram_tensor(f"dr{g}", [128, nch], f32, kind="ExternalInput")
        ext[f"ind{g}"] = nc.dram_tensor(f"ind{g}", [128, NT], f32, kind="ExternalInput")
        ext[f"outd{g}"] = nc.dram_tensor(f"outd{g}", [128, NT], f32, kind="ExternalInput")
        ext[f"odf{g}"] = nc.dram_tensor(f"odf{g}", [128, 157], f32, kind="ExternalInput")
    for nm, shp in [("W1", [D_IN, D_H]), ("W2", [D_H, D_H]), ("W3", [D_H, D_H]),
                    ("b1", [1, D_H]), ("b2", [1, D_H]), ("b3", [1, D_H]),
                    ("fW1", [D_H, 128]), ("fb1", [1, 128]), ("fW2", [128, 64]),
                    ("fb2", [1, 64]), ("fW3", [64, 1]), ("fb3", [1, 1])]:
        ext[nm] = nc.dram_tensor(nm, shp, f32, kind="ExternalInput")
    y_ext = nc.dram_tensor("y", [1, 1], f32, kind="ExternalOutput")

    iota_d = nc.inline_tensor(np.tile(np.arange(128, dtype=np.float16), (128, 1)),
                              name="iota128")
    ident_d = nc.inline_tensor(np.eye(128, dtype=np.float32), name="ident")
    ones16_d = nc.inline_tensor(np.ones((1, 128), np.float16), name="ones16")
    ones32_d = nc.inline_tensor(np.ones((1, 1), np.float32), name="ones32")
    zero_d = nc.inline_tensor(np.zeros((1, DPAD), np.float16), name="zrow")

    with tile.TileContext(nc) as tc:
        with (
            tc.tile_pool(name="cst", bufs=1) as cst,
            tc.tile_pool(name="meta", bufs=1) as meta,
            tc.tile_pool(name="g", bufs=10) as gp,
            tc.tile_pool(name="s", bufs=4) as sp,
            tc.tile_pool(name="w", bufs=3) as wp,
            tc.tile_pool(name="ps", bufs=2, space="PSUM") as pp,
            tc.tile_pool(name="ps2", bufs=2, space="PSUM") as pp2,
            tc.tile_pool(name="dram", bufs=1, space="DRAM") as dram,
        ):
            iota_t = cst.tile([128, 128], f16)
            nc.sync.dma_start(iota_t[:], iota_d[:])
            ident_t = cst.tile([128, 128], f32)
            nc.sync.dma_start(ident_t[:], ident_d[:])
            ones16 = cst.tile([1, 128], f16)
            nc.sync.dma_start(ones16[:], ones16_d[:])
            ones32 = cst.tile([1, 1], f32)
            nc.sync.dma_start(ones32[:], ones32_d[:])
            zrow = cst.tile([1, DPAD], f16)
            nc.sync.dma_start(zrow[:], zero_d[:])

            # weights resident
            W_t = {}
            w1t = cst.tile([128, D_H], f16, name="w1t")
            W_t[1] = [w1t]
            nc.gpsimd.dma_start(W_t[1][0][:], ext["W1"][:])
            for L in (2, 3):
                W_t[L] = []
                for j in range(3):
                    k = 128 if j < 2 else 48
                    w = cst.tile([128, D_H], f16, name=f"w{L}_{j}")
                    nc.gpsimd.dma_start(w[0:k, :], ext[f"W{L}"][j * 128:j * 128 + k, :])
                    W_t[L].append(w)
            b_t = {}
            for L in (1, 2, 3):
                b = cst.tile([1, D_H], f16, name=f"b{L}t")
                nc.gpsimd.dma_start(b[:], ext[f"b{L}"][:])
                b_t[L] = b
            fW1_t = []
            for j in range(3):
                k = 128 if j < 2 else 48
                w = cst.tile([128, 128], f32, name=f"fw1_{j}")
                nc.sync.dma_start(w[0:k, :], ext["fW1"][j * 128:j * 128 + k, :])
                fW1_t.append(w)
            fW2_t = cst.tile([128, 64], f32)
            nc.sync.dma_start(fW2_t[:], ext["fW2"][:])
            fW3_t = cst.tile([64, 1], f32)
            nc.sync.dma_start(fW3_t[:], ext["fW3"][:])
            fb_t = {}
            for nm, w in [("fb1", 128), ("fb2", 64), ("fb3", 1)]:
                b = cst.tile([1, w], f32, name=f"{nm}t")
                nc.sync.dma_start(b[:], ext[nm][:])
                fb_t[nm] = b

            # DRAM feature tables
            xp = dram.tile([N, D_IN], f16)
            hfA, hfB = [], []
            for g in range(3):
                ta = dram.tile([N, DPAD], f16, addr_space="Shared", name=f"hfA{g}")
                tb = dram.tile([N, DPAD], f16, addr_space="Shared", name=f"hfB{g}")
                hfA.append(ta)
                hfB.append(tb)
            shard_b = dram.tile([SH, DPAD], f16)
            pool_in = dram.tile([128, 3], f32)
            pool_out = dram.tile([128, 3], f32, addr_space="Shared")
            vec_b = dram.tile([1, 128], f32)

            macc = cst.tile([128, D_H], f32)
            nc.vector.memset(macc[:], 0.0)

            for g in range(3):
                Ck, offs_np, drel_np, _, _, _ = g_meta[g]
                nch = sum(Ck)
                off_t = meta.tile([128, nch], i32, tag="off")
                nc.sync.dma_start(off_t[:], ext[f"off{g}"][:])
                dr_t = meta.tile([128, nch], f32, tag="dr")
                nc.sync.dma_start(dr_t[:], ext[f"dr{g}"][:])
                # degree rsqrt arrays
                rind = meta.tile([128, NT], f32, tag="rind")
                tmp = meta.tile([128, NT], f32, tag="tmpd")
                nc.sync.dma_start(tmp[:], ext[f"ind{g}"][:])
                nc.scalar.sqrt(rind[:], tmp[:])
                nc.vector.reciprocal(rind[:], rind[:])
                rout = meta.tile([128, NT], f32, tag="rout")
                tmp2 = meta.tile([128, NT], f32, tag="tmpd2")
                nc.sync.dma_start(tmp2[:], ext[f"outd{g}"][:])
                nc.scalar.sqrt(rout[:], tmp2[:])
                nc.vector.reciprocal(rout[:], rout[:])
                rodf = meta.tile([128, 157], f32, tag="rodf")
                tmp3 = meta.tile([128, 157], f32, tag="tmpd3")
                nc.sync.dma_start(tmp3[:], ext[f"odf{g}"][:])
                nc.scalar.sqrt(rodf[:], tmp3[:])
                nc.vector.reciprocal(rodf[:], rodf[:])

                # x prescale -> xp (fp16)
                with nc.named_scope(f"g{g}_prescale"):
                    for t in range(157):
                        rows = 128 if t < 156 else N - 156 * 128
                        xt = gp.tile([128, D_IN], f32, tag="xt")
                        nc.sync.dma_start(xt[0:rows, :], ext[f"x{g}"][t * 128:t * 128 + rows, :])
                        xs = gp.tile([128, D_IN], f16, tag="xs")
                        nc.scalar.activation(xs[0:rows, :], xt[0:rows, :], AF.Copy,
                                             scale=rodf[0:rows, t:t + 1])
                        nc.sync.dma_start(xp[t * 128:t * 128 + rows, :], xs[0:rows, :])

                for L in (1, 2, 3):
                    src_tab = xp if L == 1 else (hfA[g] if L == 2 else hfB[g])
                    DL = D_IN if L == 1 else D_H
                    DLP = D_IN if L == 1 else DPAD
                    J = 1 if L == 1 else 3
                    j0 = 0
                    scope = nc.named_scope(f"g{g}L{L}")
                    scope.__enter__()
                    for t in range(NT):
                        rows = 128 if t < NT - 1 else SH - (NT - 1) * 128
                        psum = pp.tile([128, D_H], f32, tag="agg")
                        for cchunk in range(Ck[t]):
                            gt = gp.tile([128, DLP], f16, tag=f"g{L}")
                            nc.gpsimd.indirect_dma_start(
                                out=gt[:], out_offset=None, in_=src_tab[:],
                                in_offset=bass.IndirectOffsetOnAxis(
                                    ap=off_t[:, j0 + cchunk:j0 + cchunk + 1], axis=0))
                            s = sp.tile([128, 128], f16, tag="s")
                            nc.vector.tensor_scalar(
                                s[:], iota_t[:], dr_t[:, j0 + cchunk:j0 + cchunk + 1],
                                None, mybir.AluOpType.is_equal)
                            nc.tensor.matmul(psum[:, 0:DL], s[:], gt[:, 0:DL],
                                             start=(cchunk == 0), stop=(cchunk == Ck[t] - 1))
                        j0 += Ck[t]
                        # scale by rsqrt(indeg), transpose, W matmul
                        zsb = gp.tile([128, D_H], f32, tag="zsb")
                        nc.scalar.activation(zsb[:, 0:DL], psum[:, 0:DL], AF.Copy,
                                             scale=rind[:, t:t + 1])
                        psum2 = pp2.tile([128, D_H], f32, tag="wout")
                        for j in range(J):
                            k = 128 if (j < J - 1 or L == 1) else 48
                            tp = pp.tile([128, 128], f32, tag="tp")
                            nc.tensor.transpose(tp[0:k, :], zsb[:, j * 128:j * 128 + k],
                                                ident_t[:])
                            at = gp.tile([128, 128], f16, tag="at")
                            nc.vector.tensor_copy(at[0:k, :], tp[0:k, :])
                            nc.tensor.matmul(psum2[:], at[0:k, :], W_t[L][j][0:k, :],
                                             start=(j == 0), stop=False)
                        nc.tensor.matmul(psum2[:], ones16[:], b_t[L][:],
                                         start=False, stop=True)
                        if L < 3:
                            hsb = gp.tile([128, D_H], f16, tag="hsb")
                            nc.scalar.activation(hsb[:], psum2[:], AF.Relu,
                                                 scale=rout[:, t:t + 1])
                            nc.sync.dma_start(
                                shard_b[t * 128:t * 128 + rows, 0:D_H], hsb[0:rows, :])
                        else:
                            hsb = gp.tile([128, D_H], f32, tag="hsb3")
                            nc.scalar.activation(hsb[:], psum2[:], AF.Relu)
                            nc.vector.tensor_tensor(macc[0:rows, :], macc[0:rows, :],
                                                    hsb[0:rows, :], mybir.AluOpType.max)
                    scope.__exit__(None, None, None)
                    if L < 3:
                        dstf = hfA[g] if L == 1 else hfB[g]
                        with nc.named_scope(f"g{g}L{L}ag"):
                            nc.gpsimd.collective_compute(
                                "AllGather", mybir.AluOpType.bypass,
                                replica_groups=[core_ids],
                                ins=[shard_b.opt()],
                                outs=[dstf.opt()])

            # max over partitions via transpose + reduce, AllReduce, MLP
            scope_tail = nc.named_scope("tail")
            scope_tail.__enter__()
            pool_sb = cst.tile([128, 3], f32)
            for j in range(3):
                k = 128 if j < 2 else 48
                tp = pp.tile([128, 128], f32, tag="tp")
                nc.tensor.transpose(tp[0:k, :], macc[:, j * 128:j * 128 + k], ident_t[:])
                nc.vector.tensor_reduce(pool_sb[0:k, j:j + 1], tp[0:k, :],
                                        mybir.AxisListType.X, mybir.AluOpType.max)
            nc.sync.dma_start(pool_in[:], pool_sb[:])
            nc.gpsimd.collective_compute(
                "AllReduce", mybir.AluOpType.max, replica_groups=[core_ids],
                ins=[pool_in.opt()], outs=[pool_out.opt()])
            pool_t = cst.tile([128, 3], f32)
            nc.sync.dma_start(pool_t[:], pool_out[:])

            z1p = pp2.tile([1, 128], f32, tag="z")
            for j in range(3):
                k = 128 if j < 2 else 48
                nc.tensor.matmul(z1p[:], pool_t[0:k, j:j + 1], fW1_t[j][0:k, :],
                                 start=(j == 0), stop=False)
            nc.tensor.matmul(z1p[:], ones32[:], fb_t["fb1"][:], start=False, stop=True)
            z1s = cst.tile([1, 128], f32)
            nc.scalar.activation(z1s[:], z1p[:], AF.Relu)
            nc.sync.dma_start(vec_b[:], z1s[:])
            z1T = cst.tile([128, 1], f32)
            nc.sync.dma_start(z1T[:], vec_b[0, :].rearrange("(p o) -> p o", o=1))
            z2p = pp2.tile([1, 64], f32, tag="z")
            nc.tensor.matmul(z2p[:], z1T[:], fW2_t[:], start=True, stop=False)
            nc.tensor.matmul(z2p[:], ones32[:], fb_t["fb2"][:], start=False, stop=True)
            z2s = cst.tile([1, 64], f32)
            nc.scalar.activation(z2s[:], z2p[:], AF.Relu)
            nc.sync.dma_start(vec_b[0:1, 0:64], z2s[:])
            z2T = cst.tile([64, 1], f32)
            nc.sync.dma_start(z2T[:], vec_b[0, 0:64].rearrange("(p o) -> p o", o=1))
            z3p = pp2.tile([1, 1], f32, tag="z")
            nc.tensor.matmul(z3p[:], z2T[:], fW3_t[:], start=True, stop=False)
            nc.tensor.matmul(z3p[:], ones32[:], fb_t["fb3"][:], start=False, stop=True)
            ys = cst.tile([1, 1], f32)
            nc.scalar.activation(ys[:], z3p[:], AF.Sigmoid)
            nc.sync.dma_start(y_ext[:], ys[:])
            scope_tail.__exit__(None, None, None)

    nc.compile()
    return nc


def kernel(**inputs):
    g_meta = []
    for g, (s, d) in enumerate([("src1", "dst1"), ("src2", "dst2"), ("src3", "dst3")]):
        g_meta.append(_prep_graph(inputs[s], inputs[d]))
    nc = _build(g_meta)
    in_maps = []
    for c in range(NC_):
        m = {}
        for g, xn in enumerate(["x1", "x2", "x3"]):
            Ck, offs, drel, ind, outd, odf = g_meta[g]
            m[f"x{g}"] = np.asarray(inputs[xn], np.float32)
            m[f"off{g}"] = offs[c]
            m[f"dr{g}"] = drel[c]
            m[f"ind{g}"] = ind[c]
            m[f"outd{g}"] = outd[c]
            m[f"odf{g}"] = odf
        for nm in ["W1", "W2", "W3", "fW2"]:
            m[nm] = np.asarray(inputs[nm], np.float32)
        m["fW1"] = np.asarray(inputs["fW1"], np.float32)
        m["fW3"] = np.asarray(inputs["fW3"], np.float32).reshape(64, 1)
        for nm in ["b1", "b2", "b3", "fb1", "fb2", "fb3"]:
            m[nm] = np.asarray(inputs[nm], np.float32).reshape(1, -1)
        in_maps.append(m)
    trace = bool(int(os.environ.get("KTRACE", "0")))
    tmpdir = os.environ.get("KTRACE_DIR") or None
    res = run_bass_kernel_spmd(nc, in_maps, core_ids, trace=trace, tmpdir=tmpdir)
    kernel.last_results = res
    return np.asarray(res.results[0]["y"], np.float32).reshape(1)



# revision 17
# speedup vs baseline: 1.3004x; 1.3004x over previous
"""3-branch GCN (DGL GraphConv x3 + max-pool + MLP head) on 8 TRN2 NeuronCores.

Sharding: destination nodes (2500/core). Per layer, each core batch-gathers all
src rows for one 128-dst tile with a single dma_gather (custom SWDGE gather:
~1us fixed + 0.34ns/row), aggregates via one-hot fp16 matmuls into PSUM,
applies the dense W matmul per dst tile, and AllGathers the layer output shards
for the next layer. The three graphs are emitted interleaved so each AllGather
overlaps the other graphs' compute. Max-pool is local + AllReduce(max); the
tiny MLP head runs replicated.
"""
import os
import numpy as np
import concourse.bass as bass
import concourse.bacc as bacc
import concourse.tile as tile
import concourse.mybir as mybir
from concourse.bass_utils import run_bass_kernel_spmd

NC_ = 8
N = 20000
E = 320000
SH = N // NC_          # 2500 nodes per core
NT = 20                # dst tiles per core (19 full + 68-node partial)
D_IN, D_H = 128, 304
DPAD = 384             # fp16 row pad -> 768B rows (256B multiple for dma_gather)
f16, f32 = mybir.dt.float16, mybir.dt.float32
i32, i16 = mybir.dt.int32, mybir.dt.int16
AF = mybir.ActivationFunctionType
core_ids = list(range(NC_))


def _prep_graph(src, dst):
    """Per-core chunked edge metadata with core-uniform chunk counts."""
    src = np.asarray(src).astype(np.int64)
    dst = np.asarray(dst).astype(np.int64)
    outdeg = np.bincount(src, minlength=N).clip(1).astype(np.float32)
    indeg = np.bincount(dst, minlength=N).clip(1).astype(np.float32)
    per_core = []
    for c in range(NC_):
        m = (dst // SH) == c
        es, ed = src[m], dst[m] - c * SH
        tiles = []
        for t in range(NT):
            tm = (ed // 128) == t
            tiles.append((es[tm], ed[tm] - t * 128))
        per_core.append(tiles)
    # uniform chunk count per tile slot
    Ck = [max(int(np.ceil(len(per_core[c][t][0]) / 128)) for c in range(NC_)) or 1
          for t in range(NT)]
    nchunks = sum(Ck)
    # wrapped int16 gather indices: edge i of a tile -> [i % 16, i // 16]
    idxw = np.zeros((NC_, 128, nchunks * 8), np.int16)
    drel = np.full((NC_, 128, nchunks), -1.0, np.float32)  # -1 -> zero S col
    for c in range(NC_):
        j0 = 0
        for t in range(NT):
            es, er = per_core[c][t]
            npad = Ck[t] * 128
            e_s = np.full(npad, 0, np.int64)      # pad -> row 0 (weight 0)
            e_r = np.full(npad, -1.0, np.float32)
            e_s[:len(es)] = es
            e_r[:len(er)] = er
            drel[c, :, j0:j0 + Ck[t]] = e_r.reshape(Ck[t], 128).T
            # wrapped [16, cols] block replicated to all 8 Q7 core groups
            idxw[c, :, j0 * 8:(j0 + Ck[t]) * 8] = np.tile(
                e_s.astype(np.int16).reshape(Ck[t] * 8, 16).T, (8, 1))
            j0 += Ck[t]
    # per-core slot-ordered degree arrays [128, NT]
    ind = np.ones((NC_, 128, NT), np.float32)
    outd = np.ones((NC_, 128, NT), np.float32)
    for c in range(NC_):
        for t in range(NT):
            lo = c * SH + t * 128
            hi = min(lo + 128, (c + 1) * SH)
            ind[c, :hi - lo, t] = indeg[lo:hi]
            outd[c, :hi - lo, t] = outdeg[lo:hi]
    # full outdeg, node n -> [n % 128, n // 128] (lane-major tiles)
    tmp = np.ones(157 * 128, np.float32)
    tmp[:N] = outdeg
    odf = tmp.reshape(157, 128).T.copy()
    return Ck, idxw, drel, ind, outd, odf


def _build(g_meta):
    nc = bacc.Bacc(None, target_bir_lowering=False)
    ext = {}
    for g in range(3):
        Ck, idxw, drel, ind, outd, odf = g_meta[g]
        nch = sum(Ck)
        ext[f"x{g}"] = nc.dram_tensor(f"x{g}", [N, D_IN], f32, kind="ExternalInput")
        ext[f"ix{g}"] = nc.dram_tensor(f"ix{g}", [128, nch * 8], i16, kind="ExternalInput")
        ext[f"dr{g}"] = nc.dram_tensor(f"dr{g}", [128, nch], f32, kind="ExternalInput")
        ext[f"ind{g}"] = nc.dram_tensor(f"ind{g}", [128, NT], f32, kind="ExternalInput")
        ext[f"outd{g}"] = nc.dram_tensor(f"outd{g}", [128, NT], f32, kind="ExternalInput")
        ext[f"odf{g}"] = nc.dram_tensor(f"odf{g}", [128, 157], f32, kind="ExternalInput")
    for nm, shp in [("W1", [D_IN, D_H]), ("W2", [D_H, D_H]), ("W3", [D_H, D_H]),
                    ("b1", [1, D_H]), ("b2", [1, D_H]), ("b3", [1, D_H]),
                    ("fW1", [D_H, 128]), ("fb1", [1, 128]), ("fW2", [128, 64]),
                    ("fb2", [1, 64]), ("fW3", [64, 1]), ("fb3", [1, 1])]:
        ext[nm] = nc.dram_tensor(nm, shp, f32, kind="ExternalInput")
    y_ext = nc.dram_tensor("y", [1, 1], f32, kind="ExternalOutput")

    iota_d = nc.inline_tensor(np.tile(np.arange(128, dtype=np.float16), (128, 1)),
                              name="iota128")
    ident_d = nc.inline_tensor(np.eye(128, dtype=np.float32), name="ident")
    ones16_d = nc.inline_tensor(np.ones((1, 128), np.float16), name="ones16")
    ones32_d = nc.inline_tensor(np.ones((1, 1), np.float32), name="ones32")

    with tile.TileContext(nc) as tc:
        with (
            tc.tile_pool(name="cst", bufs=1) as cst,
            tc.tile_pool(name="meta", bufs=1) as meta,
            tc.tile_pool(name="g", bufs=3) as gp,
            tc.tile_pool(name="x", bufs=3) as xp_pool,
            tc.tile_pool(name="s", bufs=4) as sp,
            tc.tile_pool(name="o", bufs=4) as op,
            tc.tile_pool(name="ps", bufs=2, space="PSUM") as pp,
            tc.tile_pool(name="ps2", bufs=2, space="PSUM") as pp2,
            tc.tile_pool(name="dram", bufs=1, space="DRAM") as dram,
        ):
            iota_t = cst.tile([128, 128], f16)
            nc.sync.dma_start(iota_t[:], iota_d[:])
            ident_t = cst.tile([128, 128], f32)
            nc.sync.dma_start(ident_t[:], ident_d[:])
            ones16 = cst.tile([1, 128], f16)
            nc.sync.dma_start(ones16[:], ones16_d[:])
            ones32 = cst.tile([1, 1], f32)
            nc.sync.dma_start(ones32[:], ones32_d[:])

            # weights resident
            W_t = {}
            w1t = cst.tile([128, D_H], f16, name="w1t")
            W_t[1] = [w1t]
            nc.gpsimd.dma_start(W_t[1][0][:], ext["W1"][:])
            for L in (2, 3):
                W_t[L] = []
                for j in range(3):
                    k = 128 if j < 2 else 48
                    w = cst.tile([128, D_H], f16, name=f"w{L}_{j}")
                    nc.gpsimd.dma_start(w[0:k, :], ext[f"W{L}"][j * 128:j * 128 + k, :])
                    W_t[L].append(w)
            b_t = {}
            for L in (1, 2, 3):
                b = cst.tile([1, D_H], f16, name=f"b{L}t")
                nc.gpsimd.dma_start(b[:], ext[f"b{L}"][:])
                b_t[L] = b
            fW1_t = []
            for j in range(3):
                k = 128 if j < 2 else 48
                w = cst.tile([128, 128], f32, name=f"fw1_{j}")
                nc.sync.dma_start(w[0:k, :], ext["fW1"][j * 128:j * 128 + k, :])
                fW1_t.append(w)
            fW2_t = cst.tile([128, 64], f32)
            nc.sync.dma_start(fW2_t[:], ext["fW2"][:])
            fW3_t = cst.tile([64, 1], f32)
            nc.sync.dma_start(fW3_t[:], ext["fW3"][:])
            fb_t = {}
            for nm, w in [("fb1", 128), ("fb2", 64), ("fb3", 1)]:
                b = cst.tile([1, w], f32, name=f"{nm}t")
                nc.sync.dma_start(b[:], ext[nm][:])
                fb_t[nm] = b

            # per-graph metadata in SBUF
            ix_t, dr_t, rind_t, rout_t, rodf_t = {}, {}, {}, {}, {}
            for g in range(3):
                Ck = g_meta[g][0]
                nch = sum(Ck)
                ix = meta.tile([128, nch * 8], i16, name=f"ix{g}")
                nc.sync.dma_start(ix[:], ext[f"ix{g}"][:])
                ix_t[g] = ix
                dr = meta.tile([128, nch], f32, name=f"dr{g}")
                nc.sync.dma_start(dr[:], ext[f"dr{g}"][:])
                dr_t[g] = dr
                rind = meta.tile([128, NT], f32, name=f"rind{g}")
                tmp = meta.tile([128, NT], f32, tag="tmpd")
                nc.sync.dma_start(tmp[:], ext[f"ind{g}"][:])
                nc.scalar.sqrt(rind[:], tmp[:])
                nc.vector.reciprocal(rind[:], rind[:])
                rind_t[g] = rind
                rout = meta.tile([128, NT], f32, name=f"rout{g}")
                tmp2 = meta.tile([128, NT], f32, tag="tmpd2")
                nc.sync.dma_start(tmp2[:], ext[f"outd{g}"][:])
                nc.scalar.sqrt(rout[:], tmp2[:])
                nc.vector.reciprocal(rout[:], rout[:])
                rout_t[g] = rout
                rodf = meta.tile([128, 157], f32, name=f"rodf{g}")
                tmp3 = meta.tile([128, 157], f32, tag="tmpd3")
                nc.sync.dma_start(tmp3[:], ext[f"odf{g}"][:])
                nc.scalar.sqrt(rodf[:], tmp3[:])
                nc.vector.reciprocal(rodf[:], rodf[:])
                rodf_t[g] = rodf

            # DRAM feature tables
            xp = [dram.tile([N, D_IN], f16, name=f"xp{g}") for g in range(3)]
            hfA, hfB, shards = [], [], {}
            for g in range(3):
                hfA.append(dram.tile([N, DPAD], f16, addr_space="Shared", name=f"hfA{g}"))
                hfB.append(dram.tile([N, DPAD], f16, addr_space="Shared", name=f"hfB{g}"))
                shards[(g, 1)] = dram.tile([SH, DPAD], f16, name=f"sh1_{g}")
                shards[(g, 2)] = dram.tile([SH, DPAD], f16, name=f"sh2_{g}")
            pool_in = dram.tile([128, 3], f32)
            pool_out = dram.tile([128, 3], f32, addr_space="Shared")
            vec_b = dram.tile([1, 128], f32)

            macc = cst.tile([128, D_H], f32)
            nc.vector.memset(macc[:], 0.0)

            # ---- prescale: xp[g] = f16(x[g] * rsqrt(outdeg)), batched DMAs
            NB = 8  # tiles per DMA batch
            for g in range(3):
                with nc.named_scope(f"g{g}_prescale"):
                    rodf = rodf_t[g]
                    for b0 in range(0, 156, NB):
                        nb = min(NB, 156 - b0)
                        rows = nb * 128
                        r0 = b0 * 128
                        xt = xp_pool.tile([128, NB * 128], f32, tag="xt")
                        nc.sync.dma_start(
                            xt[:, 0:nb * 128].rearrange("p (c d) -> p c d", d=128),
                            ext[f"x{g}"][r0:r0 + rows, :].rearrange(
                                "(c p) d -> p c d", p=128))
                        xs = xp_pool.tile([128, NB * 128], f16, tag="xs")
                        for i in range(nb):
                            nc.scalar.activation(
                                xs[:, i * 128:(i + 1) * 128],
                                xt[:, i * 128:(i + 1) * 128], AF.Copy,
                                scale=rodf[:, b0 + i:b0 + i + 1])
                        nc.sync.dma_start(
                            xp[g][r0:r0 + rows, :].rearrange(
                                "(c p) d -> p c d", p=128),
                            xs[:, 0:nb * 128].rearrange("p (c d) -> p c d", d=128))
                    # tail tile 156: 32 rows
                    xt = xp_pool.tile([128, 128], f32, tag="xtt")
                    nc.sync.dma_start(xt[0:32, :], ext[f"x{g}"][19968:20000, :])
                    xs = xp_pool.tile([128, 128], f16, tag="xst")
                    nc.scalar.activation(xs[0:32, :], xt[0:32, :], AF.Copy,
                                         scale=rodf[0:32, 156:157])
                    nc.sync.dma_start(xp[g][19968:20000, :], xs[0:32, :])

            def layer(g, L):
                Ck = g_meta[g][0]
                src_tab = xp[g] if L == 1 else (hfA[g] if L == 2 else hfB[g])
                DL = D_IN if L == 1 else D_H
                DLP = D_IN if L == 1 else DPAD
                J = 1 if L == 1 else 3
                ix, dr = ix_t[g], dr_t[g]
                rind, rout = rind_t[g], rout_t[g]
                j0 = 0
                with nc.named_scope(f"g{g}L{L}"):
                    for t in range(NT):
                        rows = 128 if t < NT - 1 else SH - (NT - 1) * 128
                        ck = Ck[t]
                        nidx = ck * 128
                        gt = gp.tile([128, ck * DLP], f16, tag=f"g{L}")
                        nc.gpsimd.dma_gather(
                            gt[:].rearrange("p (c e) -> p c e", e=DLP),
                            src_tab[:],
                            ix[:, j0 * 8:(j0 + ck) * 8],
                            nidx, nidx, DLP, single_packet=False)
                        psum = pp.tile([128, D_H], f32, tag="agg")
                        for c in range(ck):
                            s = sp.tile([128, 128], f16, tag="s")
                            nc.vector.tensor_scalar(
                                s[:], iota_t[:], dr[:, j0 + c:j0 + c + 1],
                                None, mybir.AluOpType.is_equal)
                            nc.tensor.matmul(psum[:, 0:DL], s[:],
                                             gt[:, c * DLP:c * DLP + DL],
                                             start=(c == 0), stop=(c == ck - 1))
                        j0 += ck
                        # scale by rsqrt(indeg), transpose, W matmul
                        zsb = op.tile([128, D_H], f32, tag="zsb")
                        nc.scalar.activation(zsb[:, 0:DL], psum[:, 0:DL], AF.Copy,
                                             scale=rind[:, t:t + 1])
                        psum2 = pp2.tile([128, D_H], f32, tag="wout")
                        for j in range(J):
                            k = 128 if (j < J - 1 or L == 1) else 48
                            tp = pp.tile([128, 128], f32, tag="tp")
                            nc.tensor.transpose(tp[0:k, :], zsb[:, j * 128:j * 128 + k],
                                                ident_t[:])
                            at = op.tile([128, 128], f16, tag="at")
                            nc.vector.tensor_copy(at[0:k, :], tp[0:k, :])
                            nc.tensor.matmul(psum2[:], at[0:k, :], W_t[L][j][0:k, :],
                                             start=(j == 0), stop=False)
                        nc.tensor.matmul(psum2[:], ones16[:], b_t[L][:],
                                         start=False, stop=True)
                        if L < 3:
                            hsb = op.tile([128, D_H], f16, tag="hsb")
                            nc.scalar.activation(hsb[:], psum2[:], AF.Relu,
                                                 scale=rout[:, t:t + 1])
                            nc.sync.dma_start(
                                shards[(g, L)][t * 128:t * 128 + rows, 0:D_H],
                                hsb[0:rows, :])
                        else:
                            hsb = op.tile([128, D_H], f32, tag="hsb3")
                            nc.scalar.activation(hsb[:], psum2[:], AF.Relu)
                            nc.vector.tensor_tensor(macc[0:rows, :], macc[0:rows, :],
                                                    hsb[0:rows, :], mybir.AluOpType.max)
                if L < 3:
                    dstf = hfA[g] if L == 1 else hfB[g]
                    with nc.named_scope(f"g{g}L{L}ag"):
                        nc.gpsimd.collective_compute(
                            "AllGather", mybir.AluOpType.bypass,
                            replica_groups=[core_ids],
                            ins=[shards[(g, L)].opt()],
                            outs=[dstf.opt()])

            for g in range(3):
                layer(g, 1)
            for g in range(3):
                layer(g, 2)
            for g in range(3):
                layer(g, 3)

            # max over partitions via transpose + reduce, AllReduce, MLP
            with nc.named_scope("tail"):
                pool_sb = cst.tile([128, 3], f32)
                nc.vector.memset(pool_sb[:], 0.0)
                for j in range(3):
                    k = 128 if j < 2 else 48
                    tp = pp.tile([128, 128], f32, tag="tp")
                    nc.tensor.transpose(tp[0:k, :], macc[:, j * 128:j * 128 + k],
                                        ident_t[:])
                    nc.vector.tensor_reduce(pool_sb[0:k, j:j + 1], tp[0:k, :],
                                            mybir.AxisListType.X, mybir.AluOpType.max)
                nc.sync.dma_start(pool_in[:], pool_sb[:])
                nc.gpsimd.collective_compute(
                    "AllReduce", mybir.AluOpType.max, replica_groups=[core_ids],
                    ins=[pool_in.opt()], outs=[pool_out.opt()])
                pool_t = cst.tile([128, 3], f32)
                nc.sync.dma_start(pool_t[:], pool_out[:])

                z1p = pp2.tile([1, 128], f32, tag="z")
                for j in range(3):
                    k = 128 if j < 2 else 48
                    nc.tensor.matmul(z1p[:], pool_t[0:k, j:j + 1], fW1_t[j][0:k, :],
                                     start=(j == 0), stop=False)
                nc.tensor.matmul(z1p[:], ones32[:], fb_t["fb1"][:], start=False, stop=True)
                z1s = cst.tile([1, 128], f32)
                nc.scalar.activation(z1s[:], z1p[:], AF.Relu)
                nc.sync.dma_start(vec_b[:], z1s[:])
                z1T = cst.tile([128, 1], f32)
                nc.sync.dma_start(z1T[:], vec_b[0, :].rearrange("(p o) -> p o", o=1))
                z2p = pp2.tile([1, 64], f32, tag="z")
                nc.tensor.matmul(z2p[:], z1T[:], fW2_t[:], start=True, stop=False)
                nc.tensor.matmul(z2p[:], ones32[:], fb_t["fb2"][:], start=False, stop=True)
                z2s = cst.tile([1, 64], f32)
                nc.scalar.activation(z2s[:], z2p[:], AF.Relu)
                nc.sync.dma_start(vec_b[0:1, 0:64], z2s[:])
                z2T = cst.tile([64, 1], f32)
                nc.sync.dma_start(z2T[:], vec_b[0, 0:64].rearrange("(p o) -> p o", o=1))
                z3p = pp2.tile([1, 1], f32, tag="z")
                nc.tensor.matmul(z3p[:], z2T[:], fW3_t[:], start=True, stop=False)
                nc.tensor.matmul(z3p[:], ones32[:], fb_t["fb3"][:], start=False, stop=True)
                ys = cst.tile([1, 1], f32)
                nc.scalar.activation(ys[:], z3p[:], AF.Sigmoid)
                nc.sync.dma_start(y_ext[:], ys[:])

    nc.compile()
    return nc


def kernel(**inputs):
    g_meta = []
    for g, (s, d) in enumerate([("src1", "dst1"), ("src2", "dst2"), ("src3", "dst3")]):
        g_meta.append(_prep_graph(inputs[s], inputs[d]))
    nc = _build(g_meta)
    in_maps = []
    for c in range(NC_):
        m = {}
        for g, xn in enumerate(["x1", "x2", "x3"]):
            Ck, idxw, drel, ind, outd, odf = g_meta[g]
            m[f"x{g}"] = np.asarray(inputs[xn], np.float32)
            m[f"ix{g}"] = idxw[c]
            m[f"dr{g}"] = drel[c]
            m[f"ind{g}"] = ind[c]
            m[f"outd{g}"] = outd[c]
            m[f"odf{g}"] = odf
        for nm in ["W1", "W2", "W3", "fW2"]:
            m[nm] = np.asarray(inputs[nm], np.float32)
        m["fW1"] = np.asarray(inputs["fW1"], np.float32)
        m["fW3"] = np.asarray(inputs["fW3"], np.float32).reshape(64, 1)
        for nm in ["b1", "b2", "b3", "fb1", "fb2", "fb3"]:
            m[nm] = np.asarray(inputs[nm], np.float32).reshape(1, -1)
        in_maps.append(m)
    trace = bool(int(os.environ.get("KTRACE", "0")))
    tmpdir = os.environ.get("KTRACE_DIR") or None
    res = run_bass_kernel_spmd(nc, in_maps, core_ids, trace=trace, tmpdir=tmpdir)
    kernel.last_results = res
    return np.asarray(res.results[0]["y"], np.float32).reshape(1)


# revision 18
# speedup vs baseline: 1.3071x; 1.0052x over previous
"""3-branch GCN (DGL GraphConv x3 + max-pool + MLP head) on 8 TRN2 NeuronCores.

Sharding: destination nodes (2500/core). Per layer, each core batch-gathers all
src rows for one 128-dst tile with a single dma_gather (custom SWDGE gather:
~1us fixed + 0.34ns/row), aggregates via one-hot fp16 matmuls into PSUM,
applies the dense W matmul per dst tile, and AllGathers the layer output shards
for the next layer. The three graphs are emitted interleaved so each AllGather
overlaps the other graphs' compute. Max-pool is local + AllReduce(max); the
tiny MLP head runs replicated.
"""
import os
import numpy as np
import concourse.bass as bass
import concourse.bacc as bacc
import concourse.tile as tile
import concourse.mybir as mybir
from concourse.bass_utils import run_bass_kernel_spmd

NC_ = 8
N = 20000
E = 320000
SH = N // NC_          # 2500 nodes per core
NT = 20                # dst tiles per core (19 full + 68-node partial)
D_IN, D_H = 128, 304
DPAD = 384             # fp16 row pad -> 768B rows (256B multiple for dma_gather)
f16, f32 = mybir.dt.float16, mybir.dt.float32
i32, i16 = mybir.dt.int32, mybir.dt.int16
AF = mybir.ActivationFunctionType
core_ids = list(range(NC_))


def _prep_graph(src, dst):
    """Per-core chunked edge metadata with core-uniform chunk counts."""
    src = np.asarray(src).astype(np.int64)
    dst = np.asarray(dst).astype(np.int64)
    outdeg = np.bincount(src, minlength=N).clip(1).astype(np.float32)
    indeg = np.bincount(dst, minlength=N).clip(1).astype(np.float32)
    per_core = []
    for c in range(NC_):
        m = (dst // SH) == c
        es, ed = src[m], dst[m] - c * SH
        tiles = []
        for t in range(NT):
            tm = (ed // 128) == t
            tiles.append((es[tm], ed[tm] - t * 128))
        per_core.append(tiles)
    # uniform chunk count per tile slot
    Ck = [max(int(np.ceil(len(per_core[c][t][0]) / 128)) for c in range(NC_)) or 1
          for t in range(NT)]
    nchunks = sum(Ck)
    # wrapped int16 gather indices: edge i of a tile -> [i % 16, i // 16]
    idxw = np.zeros((NC_, 128, nchunks * 8), np.int16)
    drel = np.full((NC_, 128, nchunks), -1.0, np.float32)  # -1 -> zero S col
    for c in range(NC_):
        j0 = 0
        for t in range(NT):
            es, er = per_core[c][t]
            npad = Ck[t] * 128
            e_s = np.full(npad, 0, np.int64)      # pad -> row 0 (weight 0)
            e_r = np.full(npad, -1.0, np.float32)
            e_s[:len(es)] = es
            e_r[:len(er)] = er
            drel[c, :, j0:j0 + Ck[t]] = e_r.reshape(Ck[t], 128).T
            # wrapped [16, cols] block replicated to all 8 Q7 core groups
            idxw[c, :, j0 * 8:(j0 + Ck[t]) * 8] = np.tile(
                e_s.astype(np.int16).reshape(Ck[t] * 8, 16).T, (8, 1))
            j0 += Ck[t]
    # per-core slot-ordered degree arrays [128, NT]
    ind = np.ones((NC_, 128, NT), np.float32)
    outd = np.ones((NC_, 128, NT), np.float32)
    for c in range(NC_):
        for t in range(NT):
            lo = c * SH + t * 128
            hi = min(lo + 128, (c + 1) * SH)
            ind[c, :hi - lo, t] = indeg[lo:hi]
            outd[c, :hi - lo, t] = outdeg[lo:hi]
    # full outdeg, node n -> [n % 128, n // 128] (lane-major tiles)
    tmp = np.ones(157 * 128, np.float32)
    tmp[:N] = outdeg
    odf = tmp.reshape(157, 128).T.copy()
    return Ck, idxw, drel, ind, outd, odf


def _build(g_meta):
    nc = bacc.Bacc(None, target_bir_lowering=False)
    ext = {}
    for g in range(3):
        Ck, idxw, drel, ind, outd, odf = g_meta[g]
        nch = sum(Ck)
        ext[f"x{g}"] = nc.dram_tensor(f"x{g}", [N, D_IN], f32, kind="ExternalInput")
        ext[f"ix{g}"] = nc.dram_tensor(f"ix{g}", [128, nch * 8], i16, kind="ExternalInput")
        ext[f"dr{g}"] = nc.dram_tensor(f"dr{g}", [128, nch], f32, kind="ExternalInput")
        ext[f"ind{g}"] = nc.dram_tensor(f"ind{g}", [128, NT], f32, kind="ExternalInput")
        ext[f"outd{g}"] = nc.dram_tensor(f"outd{g}", [128, NT], f32, kind="ExternalInput")
        ext[f"odf{g}"] = nc.dram_tensor(f"odf{g}", [128, 157], f32, kind="ExternalInput")
    for nm, shp in [("W1", [D_IN, D_H]), ("W2", [D_H, D_H]), ("W3", [D_H, D_H]),
                    ("b1", [1, D_H]), ("b2", [1, D_H]), ("b3", [1, D_H]),
                    ("fW1", [D_H, 128]), ("fb1", [1, 128]), ("fW2", [128, 64]),
                    ("fb2", [1, 64]), ("fW3", [64, 1]), ("fb3", [1, 1])]:
        ext[nm] = nc.dram_tensor(nm, shp, f32, kind="ExternalInput")
    y_ext = nc.dram_tensor("y", [1, 1], f32, kind="ExternalOutput")

    iota_d = nc.inline_tensor(np.tile(np.arange(128, dtype=np.float16), (128, 1)),
                              name="iota128")
    ident_d = nc.inline_tensor(np.eye(128, dtype=np.float32), name="ident")
    ones16_d = nc.inline_tensor(np.ones((1, 128), np.float16), name="ones16")
    ones32_d = nc.inline_tensor(np.ones((1, 1), np.float32), name="ones32")

    with tile.TileContext(nc) as tc:
        with (
            tc.tile_pool(name="cst", bufs=1) as cst,
            tc.tile_pool(name="meta", bufs=1) as meta,
            tc.tile_pool(name="g", bufs=3) as gp,
            tc.tile_pool(name="x", bufs=3) as xp_pool,
            tc.tile_pool(name="s", bufs=4) as sp,
            tc.tile_pool(name="o", bufs=4) as op,
            tc.tile_pool(name="ps", bufs=2, space="PSUM") as pp,
            tc.tile_pool(name="ps2", bufs=2, space="PSUM") as pp2,
            tc.tile_pool(name="dram", bufs=1, space="DRAM") as dram,
        ):
            iota_t = cst.tile([128, 128], f16)
            nc.sync.dma_start(iota_t[:], iota_d[:])
            ident_t = cst.tile([128, 128], f32)
            nc.sync.dma_start(ident_t[:], ident_d[:])
            ones16 = cst.tile([1, 128], f16)
            nc.sync.dma_start(ones16[:], ones16_d[:])
            ones32 = cst.tile([1, 1], f32)
            nc.sync.dma_start(ones32[:], ones32_d[:])

            # weights resident
            W_t = {}
            w1t = cst.tile([128, D_H], f16, name="w1t")
            W_t[1] = [w1t]
            nc.gpsimd.dma_start(W_t[1][0][:], ext["W1"][:])
            for L in (2, 3):
                W_t[L] = []
                for j in range(3):
                    k = 128 if j < 2 else 48
                    w = cst.tile([128, D_H], f16, name=f"w{L}_{j}")
                    nc.gpsimd.dma_start(w[0:k, :], ext[f"W{L}"][j * 128:j * 128 + k, :])
                    W_t[L].append(w)
            b_t = {}
            for L in (1, 2, 3):
                b = cst.tile([1, D_H], f16, name=f"b{L}t")
                nc.gpsimd.dma_start(b[:], ext[f"b{L}"][:])
                b_t[L] = b
            fW1_t = []
            for j in range(3):
                k = 128 if j < 2 else 48
                w = cst.tile([128, 128], f32, name=f"fw1_{j}")
                nc.sync.dma_start(w[0:k, :], ext["fW1"][j * 128:j * 128 + k, :])
                fW1_t.append(w)
            fW2_t = cst.tile([128, 64], f32)
            nc.sync.dma_start(fW2_t[:], ext["fW2"][:])
            fW3_t = cst.tile([64, 1], f32)
            nc.sync.dma_start(fW3_t[:], ext["fW3"][:])
            fb_t = {}
            for nm, w in [("fb1", 128), ("fb2", 64), ("fb3", 1)]:
                b = cst.tile([1, w], f32, name=f"{nm}t")
                nc.sync.dma_start(b[:], ext[nm][:])
                fb_t[nm] = b

            # per-graph metadata in SBUF
            ix_t, dr_t, rind_t, rout_t, rodf_t = {}, {}, {}, {}, {}
            for g in range(3):
                Ck = g_meta[g][0]
                nch = sum(Ck)
                ix = meta.tile([128, nch * 8], i16, name=f"ix{g}")
                nc.sync.dma_start(ix[:], ext[f"ix{g}"][:])
                ix_t[g] = ix
                dr = meta.tile([128, nch], f32, name=f"dr{g}")
                nc.sync.dma_start(dr[:], ext[f"dr{g}"][:])
                dr_t[g] = dr
                rind = meta.tile([128, NT], f32, name=f"rind{g}")
                tmp = meta.tile([128, NT], f32, tag="tmpd")
                nc.sync.dma_start(tmp[:], ext[f"ind{g}"][:])
                nc.scalar.sqrt(rind[:], tmp[:])
                nc.vector.reciprocal(rind[:], rind[:])
                rind_t[g] = rind
                rout = meta.tile([128, NT], f32, name=f"rout{g}")
                tmp2 = meta.tile([128, NT], f32, tag="tmpd2")
                nc.sync.dma_start(tmp2[:], ext[f"outd{g}"][:])
                nc.scalar.sqrt(rout[:], tmp2[:])
                nc.vector.reciprocal(rout[:], rout[:])
                rout_t[g] = rout
                rodf = meta.tile([128, 157], f32, name=f"rodf{g}")
                tmp3 = meta.tile([128, 157], f32, tag="tmpd3")
                nc.sync.dma_start(tmp3[:], ext[f"odf{g}"][:])
                nc.scalar.sqrt(rodf[:], tmp3[:])
                nc.vector.reciprocal(rodf[:], rodf[:])
                rodf_t[g] = rodf

            # DRAM feature tables
            xp = [dram.tile([N, D_IN], f16, name=f"xp{g}") for g in range(3)]
            hfA, hfB, shards = [], [], {}
            for g in range(3):
                hfA.append(dram.tile([N, DPAD], f16, addr_space="Shared", name=f"hfA{g}"))
                hfB.append(dram.tile([N, DPAD], f16, addr_space="Shared", name=f"hfB{g}"))
                shards[(g, 1)] = dram.tile([SH, DPAD], f16, name=f"sh1_{g}")
                shards[(g, 2)] = dram.tile([SH, DPAD], f16, name=f"sh2_{g}")
            pool_in = dram.tile([128, 3], f32)
            pool_out = dram.tile([128, 3], f32, addr_space="Shared")
            vec_b = dram.tile([1, 128], f32)

            macc = cst.tile([128, D_H], f32)
            nc.vector.memset(macc[:], 0.0)

            # ---- prescale: xp[g] = f16(x[g] * rsqrt(outdeg)), batched DMAs
            NB = 8  # tiles per DMA batch
            for g in range(3):
                with nc.named_scope(f"g{g}_prescale"):
                    rodf = rodf_t[g]
                    for b0 in range(0, 156, NB):
                        nb = min(NB, 156 - b0)
                        rows = nb * 128
                        r0 = b0 * 128
                        xt = xp_pool.tile([128, NB * 128], f32, tag="xt")
                        nc.sync.dma_start(
                            xt[:, 0:nb * 128].rearrange("p (c d) -> p c d", d=128),
                            ext[f"x{g}"][r0:r0 + rows, :].rearrange(
                                "(c p) d -> p c d", p=128))
                        xs = xp_pool.tile([128, NB * 128], f16, tag="xs")
                        for i in range(nb):
                            nc.scalar.activation(
                                xs[:, i * 128:(i + 1) * 128],
                                xt[:, i * 128:(i + 1) * 128], AF.Copy,
                                scale=rodf[:, b0 + i:b0 + i + 1])
                        nc.sync.dma_start(
                            xp[g][r0:r0 + rows, :].rearrange(
                                "(c p) d -> p c d", p=128),
                            xs[:, 0:nb * 128].rearrange("p (c d) -> p c d", d=128))
                    # tail tile 156: 32 rows
                    xt = xp_pool.tile([128, 128], f32, tag="xtt")
                    nc.sync.dma_start(xt[0:32, :], ext[f"x{g}"][19968:20000, :])
                    xs = xp_pool.tile([128, 128], f16, tag="xst")
                    nc.scalar.activation(xs[0:32, :], xt[0:32, :], AF.Copy,
                                         scale=rodf[0:32, 156:157])
                    nc.sync.dma_start(xp[g][19968:20000, :], xs[0:32, :])

            def layer(g, L):
                Ck = g_meta[g][0]
                src_tab = xp[g] if L == 1 else (hfA[g] if L == 2 else hfB[g])
                DL = D_IN if L == 1 else D_H
                DLP = D_IN if L == 1 else DPAD
                J = 1 if L == 1 else 3
                ix, dr = ix_t[g], dr_t[g]
                rind, rout = rind_t[g], rout_t[g]
                j0 = 0
                with nc.named_scope(f"g{g}L{L}"):
                    for t in range(NT):
                        rows = 128 if t < NT - 1 else SH - (NT - 1) * 128
                        ck = Ck[t]
                        nidx = ck * 128
                        gt = gp.tile([128, ck * DLP], f16, tag=f"g{L}")
                        # split into <=1024-idx single-packet gathers (64-desc
                        # per-engine packet ceiling)
                        for c0 in range(0, ck, 8):
                            cn = min(8, ck - c0)
                            nc.gpsimd.dma_gather(
                                gt[:, c0 * DLP:(c0 + cn) * DLP].rearrange(
                                    "p (c e) -> p c e", e=DLP),
                                src_tab[:],
                                ix[:, (j0 + c0) * 8:(j0 + c0 + cn) * 8],
                                cn * 128, cn * 128, DLP, single_packet=True)
                        psum = pp.tile([128, D_H], f32, tag="agg")
                        for c in range(ck):
                            s = sp.tile([128, 128], f16, tag="s")
                            nc.vector.tensor_scalar(
                                s[:], iota_t[:], dr[:, j0 + c:j0 + c + 1],
                                None, mybir.AluOpType.is_equal)
                            nc.tensor.matmul(psum[:, 0:DL], s[:],
                                             gt[:, c * DLP:c * DLP + DL],
                                             start=(c == 0), stop=(c == ck - 1))
                        j0 += ck
                        # scale by rsqrt(indeg), transpose, W matmul
                        zsb = op.tile([128, D_H], f32, tag="zsb")
                        nc.scalar.activation(zsb[:, 0:DL], psum[:, 0:DL], AF.Copy,
                                             scale=rind[:, t:t + 1])
                        psum2 = pp2.tile([128, D_H], f32, tag="wout")
                        for j in range(J):
                            k = 128 if (j < J - 1 or L == 1) else 48
                            tp = pp.tile([128, 128], f32, tag="tp")
                            nc.tensor.transpose(tp[0:k, :], zsb[:, j * 128:j * 128 + k],
                                                ident_t[:])
                            at = op.tile([128, 128], f16, tag="at")
                            nc.vector.tensor_copy(at[0:k, :], tp[0:k, :])
                            nc.tensor.matmul(psum2[:], at[0:k, :], W_t[L][j][0:k, :],
                                             start=(j == 0), stop=False)
                        nc.tensor.matmul(psum2[:], ones16[:], b_t[L][:],
                                         start=False, stop=True)
                        if L < 3:
                            hsb = op.tile([128, D_H], f16, tag="hsb")
                            nc.scalar.activation(hsb[:], psum2[:], AF.Relu,
                                                 scale=rout[:, t:t + 1])
                            nc.sync.dma_start(
                                shards[(g, L)][t * 128:t * 128 + rows, 0:D_H],
                                hsb[0:rows, :])
                        else:
                            hsb = op.tile([128, D_H], f32, tag="hsb3")
                            nc.scalar.activation(hsb[:], psum2[:], AF.Relu)
                            nc.vector.tensor_tensor(macc[0:rows, :], macc[0:rows, :],
                                                    hsb[0:rows, :], mybir.AluOpType.max)
                if L < 3:
                    dstf = hfA[g] if L == 1 else hfB[g]
                    with nc.named_scope(f"g{g}L{L}ag"):
                        nc.gpsimd.collective_compute(
                            "AllGather", mybir.AluOpType.bypass,
                            replica_groups=[core_ids],
                            ins=[shards[(g, L)].opt()],
                            outs=[dstf.opt()])

            for g in range(3):
                layer(g, 1)
            for g in range(3):
                layer(g, 2)
            for g in range(3):
                layer(g, 3)

            # max over partitions via transpose + reduce, AllReduce, MLP
            with nc.named_scope("tail"):
                pool_sb = cst.tile([128, 3], f32)
                nc.vector.memset(pool_sb[:], 0.0)
                for j in range(3):
                    k = 128 if j < 2 else 48
                    tp = pp.tile([128, 128], f32, tag="tp")
                    nc.tensor.transpose(tp[0:k, :], macc[:, j * 128:j * 128 + k],
                                        ident_t[:])
                    nc.vector.tensor_reduce(pool_sb[0:k, j:j + 1], tp[0:k, :],
                                            mybir.AxisListType.X, mybir.AluOpType.max)
                nc.sync.dma_start(pool_in[:], pool_sb[:])
                nc.gpsimd.collective_compute(
                    "AllReduce", mybir.AluOpType.max, replica_groups=[core_ids],
                    ins=[pool_in.opt()], outs=[pool_out.opt()])
                pool_t = cst.tile([128, 3], f32)
                nc.sync.dma_start(pool_t[:], pool_out[:])

                z1p = pp2.tile([1, 128], f32, tag="z")
                for j in range(3):
                    k = 128 if j < 2 else 48
                    nc.tensor.matmul(z1p[:], pool_t[0:k, j:j + 1], fW1_t[j][0:k, :],
                                     start=(j == 0), stop=False)
                nc.tensor.matmul(z1p[:], ones32[:], fb_t["fb1"][:], start=False, stop=True)
                z1s = cst.tile([1, 128], f32)
                nc.scalar.activation(z1s[:], z1p[:], AF.Relu)
                nc.sync.dma_start(vec_b[:], z1s[:])
                z1T = cst.tile([128, 1], f32)
                nc.sync.dma_start(z1T[:], vec_b[0, :].rearrange("(p o) -> p o", o=1))
                z2p = pp2.tile([1, 64], f32, tag="z")
                nc.tensor.matmul(z2p[:], z1T[:], fW2_t[:], start=True, stop=False)
                nc.tensor.matmul(z2p[:], ones32[:], fb_t["fb2"][:], start=False, stop=True)
                z2s = cst.tile([1, 64], f32)
                nc.scalar.activation(z2s[:], z2p[:], AF.Relu)
                nc.sync.dma_start(vec_b[0:1, 0:64], z2s[:])
                z2T = cst.tile([64, 1], f32)
                nc.sync.dma_start(z2T[:], vec_b[0, 0:64].rearrange("(p o) -> p o", o=1))
                z3p = pp2.tile([1, 1], f32, tag="z")
                nc.tensor.matmul(z3p[:], z2T[:], fW3_t[:], start=True, stop=False)
                nc.tensor.matmul(z3p[:], ones32[:], fb_t["fb3"][:], start=False, stop=True)
                ys = cst.tile([1, 1], f32)
                nc.scalar.activation(ys[:], z3p[:], AF.Sigmoid)
                nc.sync.dma_start(y_ext[:], ys[:])

    nc.compile()
    return nc


def kernel(**inputs):
    g_meta = []
    for g, (s, d) in enumerate([("src1", "dst1"), ("src2", "dst2"), ("src3", "dst3")]):
        g_meta.append(_prep_graph(inputs[s], inputs[d]))
    nc = _build(g_meta)
    in_maps = []
    for c in range(NC_):
        m = {}
        for g, xn in enumerate(["x1", "x2", "x3"]):
            Ck, idxw, drel, ind, outd, odf = g_meta[g]
            m[f"x{g}"] = np.asarray(inputs[xn], np.float32)
            m[f"ix{g}"] = idxw[c]
            m[f"dr{g}"] = drel[c]
            m[f"ind{g}"] = ind[c]
            m[f"outd{g}"] = outd[c]
            m[f"odf{g}"] = odf
        for nm in ["W1", "W2", "W3", "fW2"]:
            m[nm] = np.asarray(inputs[nm], np.float32)
        m["fW1"] = np.asarray(inputs["fW1"], np.float32)
        m["fW3"] = np.asarray(inputs["fW3"], np.float32).reshape(64, 1)
        for nm in ["b1", "b2", "b3", "fb1", "fb2", "fb3"]:
            m[nm] = np.asarray(inputs[nm], np.float32).reshape(1, -1)
        in_maps.append(m)
    trace = bool(int(os.environ.get("KTRACE", "0")))
    tmpdir = os.environ.get("KTRACE_DIR") or None
    res = run_bass_kernel_spmd(nc, in_maps, core_ids, trace=trace, tmpdir=tmpdir)
    kernel.last_results = res
    return np.asarray(res.results[0]["y"], np.float32).reshape(1)


# revision 27
# speedup vs baseline: 1.3584x; 1.0393x over previous
"""3-branch GCN (DGL GraphConv x3 + max-pool + MLP head) on 8 TRN2 NeuronCores.

Sharding: destination nodes (2500/core). Per layer, each core batch-gathers all
src rows for one 128-dst tile with a single dma_gather (custom SWDGE gather:
~1us fixed + 0.34ns/row), aggregates via one-hot fp16 matmuls into PSUM,
applies the dense W matmul per dst tile, and AllGathers the layer output shards
for the next layer. The three graphs are emitted interleaved so each AllGather
overlaps the other graphs' compute. Max-pool is local + AllReduce(max); the
tiny MLP head runs replicated.
"""
import os
import numpy as np
import concourse.bass as bass
import concourse.bacc as bacc
import concourse.tile as tile
import concourse.mybir as mybir
from concourse.bass_utils import run_bass_kernel_spmd

NC_ = 8
N = 20000
E = 320000
SH = N // NC_          # 2500 nodes per core
NT = 20                # dst tiles per core (19 full + 68-node partial)
D_IN, D_H = 128, 304
DPAD = 384             # fp16 row pad -> 768B rows (256B multiple for dma_gather)
f16, f32 = mybir.dt.float16, mybir.dt.float32
i32, i16 = mybir.dt.int32, mybir.dt.int16
AF = mybir.ActivationFunctionType
core_ids = list(range(NC_))


def _prep_graph(src, dst):
    """Per-core chunked edge metadata with core-uniform chunk counts."""
    src = np.asarray(src).astype(np.int64)
    dst = np.asarray(dst).astype(np.int64)
    outdeg = np.bincount(src, minlength=N).clip(1).astype(np.float32)
    indeg = np.bincount(dst, minlength=N).clip(1).astype(np.float32)
    per_core = []
    for c in range(NC_):
        m = (dst // SH) == c
        es, ed = src[m], dst[m] - c * SH
        tiles = []
        for t in range(NT):
            tm = (ed // 128) == t
            tiles.append((es[tm], ed[tm] - t * 128))
        per_core.append(tiles)
    # uniform chunk count per tile slot
    Ck = [max(int(np.ceil(len(per_core[c][t][0]) / 128)) for c in range(NC_)) or 1
          for t in range(NT)]
    nchunks = sum(Ck)
    # wrapped int16 gather indices: edge i of a tile -> [i % 16, i // 16]
    idxw = np.zeros((NC_, 128, nchunks * 8), np.int16)
    drel = np.full((NC_, 128, nchunks), -1.0, np.float32)  # -1 -> zero S col
    for c in range(NC_):
        j0 = 0
        for t in range(NT):
            es, er = per_core[c][t]
            npad = Ck[t] * 128
            e_s = np.full(npad, 0, np.int64)      # pad -> row 0 (weight 0)
            e_r = np.full(npad, -1.0, np.float32)
            e_s[:len(es)] = es
            e_r[:len(er)] = er
            drel[c, :, j0:j0 + Ck[t]] = e_r.reshape(Ck[t], 128).T
            # wrapped [16, cols] block replicated to all 8 Q7 core groups
            idxw[c, :, j0 * 8:(j0 + Ck[t]) * 8] = np.tile(
                e_s.astype(np.int16).reshape(Ck[t] * 8, 16).T, (8, 1))
            j0 += Ck[t]
    # per-core slot-ordered degree arrays [128, NT]
    ind = np.ones((NC_, 128, NT), np.float32)
    outd = np.ones((NC_, 128, NT), np.float32)
    for c in range(NC_):
        for t in range(NT):
            lo = c * SH + t * 128
            hi = min(lo + 128, (c + 1) * SH)
            ind[c, :hi - lo, t] = indeg[lo:hi]
            outd[c, :hi - lo, t] = outdeg[lo:hi]
    # full outdeg, node n -> [n % 128, n // 128] (lane-major tiles)
    tmp = np.ones(157 * 128, np.float32)
    tmp[:N] = outdeg
    odf = tmp.reshape(157, 128).T.copy()
    # host-precomputed one-hot S matrices: smat[:, j*128+m] = (drel[:, j] == m)
    smat = (drel[:, :, :, None] ==
            np.arange(128, dtype=np.float32)[None, None, None, :]
            ).astype(np.float16).reshape(NC_, 128, nchunks * 128)
    return Ck, idxw, smat, ind, outd, odf


def _build(g_meta):
    nc = bacc.Bacc(None, target_bir_lowering=False)
    ext = {}
    for g in range(3):
        Ck, idxw, drel, ind, outd, odf = g_meta[g]
        nch = sum(Ck)
        ext[f"x{g}"] = nc.dram_tensor(f"x{g}", [N, D_IN], f32, kind="ExternalInput")
        ext[f"ix{g}"] = nc.dram_tensor(f"ix{g}", [128, nch * 8], i16, kind="ExternalInput")
        ext[f"S{g}"] = nc.dram_tensor(f"S{g}", [128, nch * 128], f16, kind="ExternalInput")
        ext[f"ind{g}"] = nc.dram_tensor(f"ind{g}", [128, NT], f32, kind="ExternalInput")
        ext[f"outd{g}"] = nc.dram_tensor(f"outd{g}", [128, NT], f32, kind="ExternalInput")
        ext[f"odf{g}"] = nc.dram_tensor(f"odf{g}", [128, 157], f32, kind="ExternalInput")
    for nm, shp in [("W1", [D_IN, D_H]), ("W2", [D_H, D_H]), ("W3", [D_H, D_H]),
                    ("b1", [1, D_H]), ("b2", [1, D_H]), ("b3", [1, D_H]),
                    ("fW1", [D_H, 128]), ("fb1", [1, 128]), ("fW2", [128, 64]),
                    ("fb2", [1, 64]), ("fW3", [64, 1]), ("fb3", [1, 1])]:
        ext[nm] = nc.dram_tensor(nm, shp, f32, kind="ExternalInput")
    y_ext = nc.dram_tensor("y", [1, 1], f32, kind="ExternalOutput")

    ident_d = nc.inline_tensor(np.eye(128, dtype=np.float32), name="ident")
    ones16_d = nc.inline_tensor(np.ones((1, 128), np.float16), name="ones16")
    ones32_d = nc.inline_tensor(np.ones((1, 1), np.float32), name="ones32")

    with tile.TileContext(nc) as tc:
        with (
            tc.tile_pool(name="cst", bufs=1) as cst,
            tc.tile_pool(name="meta", bufs=1) as meta,
            tc.tile_pool(name="g", bufs=3) as gp,
            tc.tile_pool(name="x", bufs=3) as xp_pool,
            tc.tile_pool(name="s", bufs=4) as sp,
            tc.tile_pool(name="o", bufs=4) as op,
            tc.tile_pool(name="ps", bufs=2, space="PSUM") as pp,
            tc.tile_pool(name="ps2", bufs=2, space="PSUM") as pp2,
            tc.tile_pool(name="dram", bufs=1, space="DRAM") as dram,
        ):
            ident_t = cst.tile([128, 128], f32)
            nc.sync.dma_start(ident_t[:], ident_d[:])
            ones16 = cst.tile([1, 128], f16)
            nc.sync.dma_start(ones16[:], ones16_d[:])
            ones32 = cst.tile([1, 1], f32)
            nc.sync.dma_start(ones32[:], ones32_d[:])

            # weights resident
            W_t = {}
            w1t = cst.tile([128, D_H], f16, name="w1t")
            W_t[1] = [w1t]
            nc.gpsimd.dma_start(W_t[1][0][:], ext["W1"][:])
            for L in (2, 3):
                W_t[L] = []
                for j in range(3):
                    k = 128 if j < 2 else 48
                    w = cst.tile([128, D_H], f16, name=f"w{L}_{j}")
                    nc.gpsimd.dma_start(w[0:k, :], ext[f"W{L}"][j * 128:j * 128 + k, :])
                    W_t[L].append(w)
            b_t = {}
            for L in (1, 2, 3):
                b = cst.tile([1, D_H], f16, name=f"b{L}t")
                nc.gpsimd.dma_start(b[:], ext[f"b{L}"][:])
                b_t[L] = b
            fW1_t = []
            for j in range(3):
                k = 128 if j < 2 else 48
                w = cst.tile([128, 128], f32, name=f"fw1_{j}")
                nc.sync.dma_start(w[0:k, :], ext["fW1"][j * 128:j * 128 + k, :])
                fW1_t.append(w)
            fW2_t = cst.tile([128, 64], f32)
            nc.sync.dma_start(fW2_t[:], ext["fW2"][:])
            fW3_t = cst.tile([64, 1], f32)
            nc.sync.dma_start(fW3_t[:], ext["fW3"][:])
            fb_t = {}
            for nm, w in [("fb1", 128), ("fb2", 64), ("fb3", 1)]:
                b = cst.tile([1, w], f32, name=f"{nm}t")
                nc.sync.dma_start(b[:], ext[nm][:])
                fb_t[nm] = b

            # per-graph metadata in SBUF
            ix_t, rind_t, rout_t, rodf_t = {}, {}, {}, {}
            for g in range(3):
                Ck = g_meta[g][0]
                nch = sum(Ck)
                ix = meta.tile([128, nch * 8], i16, name=f"ix{g}")
                nc.sync.dma_start(ix[:], ext[f"ix{g}"][:])
                ix_t[g] = ix
                rind = meta.tile([128, NT], f32, name=f"rind{g}")
                tmp = meta.tile([128, NT], f32, tag="tmpd")
                nc.sync.dma_start(tmp[:], ext[f"ind{g}"][:])
                nc.scalar.sqrt(rind[:], tmp[:])
                nc.vector.reciprocal(rind[:], rind[:])
                rind_t[g] = rind
                rout = meta.tile([128, NT], f32, name=f"rout{g}")
                tmp2 = meta.tile([128, NT], f32, tag="tmpd2")
                nc.sync.dma_start(tmp2[:], ext[f"outd{g}"][:])
                nc.scalar.sqrt(rout[:], tmp2[:])
                nc.vector.reciprocal(rout[:], rout[:])
                rout_t[g] = rout
                rodf = meta.tile([128, 157], f32, name=f"rodf{g}")
                tmp3 = meta.tile([128, 157], f32, tag="tmpd3")
                nc.sync.dma_start(tmp3[:], ext[f"odf{g}"][:])
                nc.scalar.sqrt(rodf[:], tmp3[:])
                nc.vector.reciprocal(rodf[:], rodf[:])
                rodf_t[g] = rodf

            # DRAM feature tables
            xp = [dram.tile([N, D_IN], f16, name=f"xp{g}") for g in range(3)]
            hfA, hfB, shards = [], [], {}
            for g in range(3):
                hfA.append(dram.tile([N, DPAD], f16, addr_space="Shared", name=f"hfA{g}"))
                hfB.append(dram.tile([N, DPAD], f16, addr_space="Shared", name=f"hfB{g}"))
                shards[(g, 1)] = dram.tile([SH, DPAD], f16, name=f"sh1_{g}")
                shards[(g, 2)] = dram.tile([SH, DPAD], f16, name=f"sh2_{g}")
            pool_in = dram.tile([128, 3], f32)
            pool_out = dram.tile([128, 3], f32, addr_space="Shared")
            vec_b = dram.tile([1, 128], f32)

            macc = cst.tile([128, D_H], f32)
            nc.vector.memset(macc[:], 0.0)

            # ---- prescale: xp[g] = f16(x[g] * rsqrt(outdeg)), batched DMAs
            NB = 8  # tiles per DMA batch
            for g in range(3):
                with nc.named_scope(f"g{g}_prescale"):
                    rodf = rodf_t[g]
                    for b0 in range(0, 156, NB):
                        nb = min(NB, 156 - b0)
                        rows = nb * 128
                        r0 = b0 * 128
                        xt = xp_pool.tile([128, NB * 128], f32, tag="xt")
                        nc.sync.dma_start(
                            xt[:, 0:nb * 128].rearrange("p (c d) -> p c d", d=128),
                            ext[f"x{g}"][r0:r0 + rows, :].rearrange(
                                "(c p) d -> p c d", p=128))
                        xs = xp_pool.tile([128, NB * 128], f16, tag="xs")
                        for i in range(nb):
                            nc.scalar.activation(
                                xs[:, i * 128:(i + 1) * 128],
                                xt[:, i * 128:(i + 1) * 128], AF.Copy,
                                scale=rodf[:, b0 + i:b0 + i + 1])
                        nc.sync.dma_start(
                            xp[g][r0:r0 + rows, :].rearrange(
                                "(c p) d -> p c d", p=128),
                            xs[:, 0:nb * 128].rearrange("p (c d) -> p c d", d=128))
                    # tail tile 156: 32 rows
                    xt = xp_pool.tile([128, 128], f32, tag="xtt")
                    nc.sync.dma_start(xt[0:32, :], ext[f"x{g}"][19968:20000, :])
                    xs = xp_pool.tile([128, 128], f16, tag="xst")
                    nc.scalar.activation(xs[0:32, :], xt[0:32, :], AF.Copy,
                                         scale=rodf[0:32, 156:157])
                    nc.sync.dma_start(xp[g][19968:20000, :], xs[0:32, :])

            def layer(g, L):
                Ck = g_meta[g][0]
                src_tab = xp[g] if L == 1 else (hfA[g] if L == 2 else hfB[g])
                DL = D_IN if L == 1 else D_H
                DLP = D_IN if L == 1 else DPAD
                J = 1 if L == 1 else 3
                ix = ix_t[g]
                rind, rout = rind_t[g], rout_t[g]
                j0 = 0
                with nc.named_scope(f"g{g}L{L}"):
                    for t in range(NT):
                        rows = 128 if t < NT - 1 else SH - (NT - 1) * 128
                        ck = Ck[t]
                        nidx = ck * 128
                        gt = gp.tile([128, ck * DLP], f16, tag=f"g{L}")
                        psum = pp.tile([128, D_H], f32, tag="agg")
                        # 8-chunk groups: <=1024-idx single-packet gathers
                        # (64-desc per-engine packet ceiling) + host S stream
                        for c0 in range(0, ck, 8):
                            cn = min(8, ck - c0)
                            nc.gpsimd.dma_gather(
                                gt[:, c0 * DLP:(c0 + cn) * DLP].rearrange(
                                    "p (c e) -> p c e", e=DLP),
                                src_tab[:],
                                ix[:, (j0 + c0) * 8:(j0 + c0 + cn) * 8],
                                cn * 128, cn * 128, DLP, single_packet=True)
                            st = sp.tile([128, 8 * 128], f16, tag="st")
                            nc.sync.dma_start(
                                st[:, 0:cn * 128],
                                ext[f"S{g}"][:, (j0 + c0) * 128:(j0 + c0 + cn) * 128])
                            for c in range(cn):
                                cc = c0 + c
                                nc.tensor.matmul(psum[:, 0:DL],
                                                 st[:, c * 128:(c + 1) * 128],
                                                 gt[:, cc * DLP:cc * DLP + DL],
                                                 start=(cc == 0), stop=(cc == ck - 1))
                        j0 += ck
                        # scale by rsqrt(indeg), transpose, W matmul
                        zsb = op.tile([128, D_H], f32, tag="zsb")
                        nc.scalar.activation(zsb[:, 0:DL], psum[:, 0:DL], AF.Copy,
                                             scale=rind[:, t:t + 1])
                        psum2 = pp2.tile([128, D_H], f32, tag="wout")
                        for j in range(J):
                            k = 128 if (j < J - 1 or L == 1) else 48
                            tp = pp.tile([128, 128], f32, tag="tp")
                            nc.tensor.transpose(tp[0:k, :], zsb[:, j * 128:j * 128 + k],
                                                ident_t[:])
                            at = op.tile([128, 128], f16, tag="at")
                            nc.scalar.activation(at[0:k, :], tp[0:k, :], AF.Copy)
                            nc.tensor.matmul(psum2[:], at[0:k, :], W_t[L][j][0:k, :],
                                             start=(j == 0), stop=False)
                        nc.tensor.matmul(psum2[:], ones16[:], b_t[L][:],
                                         start=False, stop=True)
                        if L < 3:
                            hsb = op.tile([128, D_H], f16, tag="hsb")
                            nc.scalar.activation(hsb[:], psum2[:], AF.Relu,
                                                 scale=rout[:, t:t + 1])
                            nc.sync.dma_start(
                                shards[(g, L)][t * 128:t * 128 + rows, 0:D_H],
                                hsb[0:rows, :])
                        else:
                            hsb = op.tile([128, D_H], f32, tag="hsb3")
                            nc.scalar.activation(hsb[:], psum2[:], AF.Relu)
                            nc.vector.tensor_tensor(macc[0:rows, :], macc[0:rows, :],
                                                    hsb[0:rows, :], mybir.AluOpType.max)
                if L < 3:
                    dstf = hfA[g] if L == 1 else hfB[g]
                    with nc.named_scope(f"g{g}L{L}ag"):
                        nc.gpsimd.collective_compute(
                            "AllGather", mybir.AluOpType.bypass,
                            replica_groups=[core_ids],
                            ins=[shards[(g, L)].opt()],
                            outs=[dstf.opt()])

            for g in range(3):
                layer(g, 1)
            for g in range(3):
                layer(g, 2)
            for g in range(3):
                layer(g, 3)

            # max over partitions via transpose + reduce, AllReduce, MLP
            with nc.named_scope("tail"):
                pool_sb = cst.tile([128, 3], f32)
                nc.vector.memset(pool_sb[:], 0.0)
                for j in range(3):
                    k = 128 if j < 2 else 48
                    tp = pp.tile([128, 128], f32, tag="tp")
                    nc.tensor.transpose(tp[0:k, :], macc[:, j * 128:j * 128 + k],
                                        ident_t[:])
                    nc.vector.tensor_reduce(pool_sb[0:k, j:j + 1], tp[0:k, :],
                                            mybir.AxisListType.X, mybir.AluOpType.max)
                nc.sync.dma_start(pool_in[:], pool_sb[:])
                nc.gpsimd.collective_compute(
                    "AllReduce", mybir.AluOpType.max, replica_groups=[core_ids],
                    ins=[pool_in.opt()], outs=[pool_out.opt()])
                pool_t = cst.tile([128, 3], f32)
                nc.sync.dma_start(pool_t[:], pool_out[:])

                z1p = pp2.tile([1, 128], f32, tag="z")
                for j in range(3):
                    k = 128 if j < 2 else 48
                    nc.tensor.matmul(z1p[:], pool_t[0:k, j:j + 1], fW1_t[j][0:k, :],
                                     start=(j == 0), stop=False)
                nc.tensor.matmul(z1p[:], ones32[:], fb_t["fb1"][:], start=False, stop=True)
                z1s = cst.tile([1, 128], f32)
                nc.scalar.activation(z1s[:], z1p[:], AF.Relu)
                nc.sync.dma_start(vec_b[:], z1s[:])
                z1T = cst.tile([128, 1], f32)
                nc.sync.dma_start(z1T[:], vec_b[0, :].rearrange("(p o) -> p o", o=1))
                z2p = pp2.tile([1, 64], f32, tag="z")
                nc.tensor.matmul(z2p[:], z1T[:], fW2_t[:], start=True, stop=False)
                nc.tensor.matmul(z2p[:], ones32[:], fb_t["fb2"][:], start=False, stop=True)
                z2s = cst.tile([1, 64], f32)
                nc.scalar.activation(z2s[:], z2p[:], AF.Relu)
                nc.sync.dma_start(vec_b[0:1, 0:64], z2s[:])
                z2T = cst.tile([64, 1], f32)
                nc.sync.dma_start(z2T[:], vec_b[0, 0:64].rearrange("(p o) -> p o", o=1))
                z3p = pp2.tile([1, 1], f32, tag="z")
                nc.tensor.matmul(z3p[:], z2T[:], fW3_t[:], start=True, stop=False)
                nc.tensor.matmul(z3p[:], ones32[:], fb_t["fb3"][:], start=False, stop=True)
                ys = cst.tile([1, 1], f32)
                nc.scalar.activation(ys[:], z3p[:], AF.Sigmoid)
                nc.sync.dma_start(y_ext[:], ys[:])

    nc.compile()
    return nc


def kernel(**inputs):
    g_meta = []
    for g, (s, d) in enumerate([("src1", "dst1"), ("src2", "dst2"), ("src3", "dst3")]):
        g_meta.append(_prep_graph(inputs[s], inputs[d]))
    nc = _build(g_meta)
    in_maps = []
    for c in range(NC_):
        m = {}
        for g, xn in enumerate(["x1", "x2", "x3"]):
            Ck, idxw, smat, ind, outd, odf = g_meta[g]
            m[f"x{g}"] = np.asarray(inputs[xn], np.float32)
            m[f"ix{g}"] = idxw[c]
            m[f"S{g}"] = smat[c]
            m[f"ind{g}"] = ind[c]
            m[f"outd{g}"] = outd[c]
            m[f"odf{g}"] = odf
        for nm in ["W1", "W2", "W3", "fW2"]:
            m[nm] = np.asarray(inputs[nm], np.float32)
        m["fW1"] = np.asarray(inputs["fW1"], np.float32)
        m["fW3"] = np.asarray(inputs["fW3"], np.float32).reshape(64, 1)
        for nm in ["b1", "b2", "b3", "fb1", "fb2", "fb3"]:
            m[nm] = np.asarray(inputs[nm], np.float32).reshape(1, -1)
        in_maps.append(m)
    trace = bool(int(os.environ.get("KTRACE", "0")))
    tmpdir = os.environ.get("KTRACE_DIR") or None
    res = run_bass_kernel_spmd(nc, in_maps, core_ids, trace=trace, tmpdir=tmpdir)
    kernel.last_results = res
    return np.asarray(res.results[0]["y"], np.float32).reshape(1)


# revision 28
# speedup vs baseline: 1.4361x; 1.0572x over previous
"""3-branch GCN (DGL GraphConv x3 + max-pool + MLP head) on 8 TRN2 NeuronCores.

Sharding: destination nodes (2500/core). Per layer, each core batch-gathers all
src rows for one 128-dst tile with a single dma_gather (custom SWDGE gather:
~1us fixed + 0.34ns/row), aggregates via one-hot fp16 matmuls into PSUM,
applies the dense W matmul per dst tile, and AllGathers the layer output shards
for the next layer. The three graphs are emitted interleaved so each AllGather
overlaps the other graphs' compute. Max-pool is local + AllReduce(max); the
tiny MLP head runs replicated.
"""
import os
import numpy as np
import concourse.bass as bass
import concourse.bacc as bacc
import concourse.tile as tile
import concourse.mybir as mybir
from concourse.bass_utils import run_bass_kernel_spmd

NC_ = 8
N = 20000
E = 320000
SH = N // NC_          # 2500 nodes per core
NT = 20                # dst tiles per core (19 full + 68-node partial)
D_IN, D_H = 128, 304
DPAD = 384             # fp16 row pad -> 768B rows (256B multiple for dma_gather)
f16, f32 = mybir.dt.float16, mybir.dt.float32
i32, i16 = mybir.dt.int32, mybir.dt.int16
AF = mybir.ActivationFunctionType
core_ids = list(range(NC_))


def _prep_graph(src, dst):
    """Per-core chunked edge metadata with core-uniform chunk counts."""
    src = np.asarray(src).astype(np.int64)
    dst = np.asarray(dst).astype(np.int64)
    outdeg = np.bincount(src, minlength=N).clip(1).astype(np.float32)
    indeg = np.bincount(dst, minlength=N).clip(1).astype(np.float32)
    per_core = []
    for c in range(NC_):
        m = (dst // SH) == c
        es, ed = src[m], dst[m] - c * SH
        tiles = []
        for t in range(NT):
            tm = (ed // 128) == t
            tiles.append((es[tm], ed[tm] - t * 128))
        per_core.append(tiles)
    # uniform chunk count per tile slot
    Ck = [max(int(np.ceil(len(per_core[c][t][0]) / 128)) for c in range(NC_)) or 1
          for t in range(NT)]
    nchunks = sum(Ck)
    # wrapped int16 gather indices: edge i of a tile -> [i % 16, i // 16]
    idxw = np.zeros((NC_, 128, nchunks * 8), np.int16)
    drel = np.full((NC_, 128, nchunks), -1.0, np.float32)  # -1 -> zero S col
    for c in range(NC_):
        j0 = 0
        for t in range(NT):
            es, er = per_core[c][t]
            npad = Ck[t] * 128
            e_s = np.full(npad, 0, np.int64)      # pad -> row 0 (weight 0)
            e_r = np.full(npad, -1.0, np.float32)
            e_s[:len(es)] = es
            e_r[:len(er)] = er
            drel[c, :, j0:j0 + Ck[t]] = e_r.reshape(Ck[t], 128).T
            # wrapped [16, cols] block replicated to all 8 Q7 core groups
            idxw[c, :, j0 * 8:(j0 + Ck[t]) * 8] = np.tile(
                e_s.astype(np.int16).reshape(Ck[t] * 8, 16).T, (8, 1))
            j0 += Ck[t]
    # per-core slot-ordered degree arrays [128, NT]
    ind = np.ones((NC_, 128, NT), np.float32)
    outd = np.ones((NC_, 128, NT), np.float32)
    for c in range(NC_):
        for t in range(NT):
            lo = c * SH + t * 128
            hi = min(lo + 128, (c + 1) * SH)
            ind[c, :hi - lo, t] = indeg[lo:hi]
            outd[c, :hi - lo, t] = outdeg[lo:hi]
    # full outdeg, node n -> [n % 128, n // 128] (lane-major tiles)
    tmp = np.ones(157 * 128, np.float32)
    tmp[:N] = outdeg
    odf = tmp.reshape(157, 128).T.copy()
    # host-precomputed one-hot S matrices: smat[:, j*128+m] = (drel[:, j] == m)
    smat = (drel[:, :, :, None] ==
            np.arange(128, dtype=np.float32)[None, None, None, :]
            ).astype(np.float16).reshape(NC_, 128, nchunks * 128)
    return Ck, idxw, smat, ind, outd, odf


def _build(g_meta):
    nc = bacc.Bacc(None, target_bir_lowering=False)
    ext = {}
    for g in range(3):
        Ck, idxw, drel, ind, outd, odf = g_meta[g]
        nch = sum(Ck)
        ext[f"x{g}"] = nc.dram_tensor(f"x{g}", [N, D_IN], f32, kind="ExternalInput")
        ext[f"ix{g}"] = nc.dram_tensor(f"ix{g}", [128, nch * 8], i16, kind="ExternalInput")
        ext[f"S{g}"] = nc.dram_tensor(f"S{g}", [128, nch * 128], f16, kind="ExternalInput")
        ext[f"ind{g}"] = nc.dram_tensor(f"ind{g}", [128, NT], f32, kind="ExternalInput")
        ext[f"outd{g}"] = nc.dram_tensor(f"outd{g}", [128, NT], f32, kind="ExternalInput")
        ext[f"odf{g}"] = nc.dram_tensor(f"odf{g}", [128, 157], f32, kind="ExternalInput")
    for nm, shp in [("W1", [D_IN, D_H]), ("W2", [D_H, D_H]), ("W3", [D_H, D_H]),
                    ("b1", [1, D_H]), ("b2", [1, D_H]), ("b3", [1, D_H]),
                    ("fW1", [D_H, 128]), ("fb1", [1, 128]), ("fW2", [128, 64]),
                    ("fb2", [1, 64]), ("fW3", [64, 1]), ("fb3", [1, 1])]:
        ext[nm] = nc.dram_tensor(nm, shp, f32, kind="ExternalInput")
    y_ext = nc.dram_tensor("y", [1, 1], f32, kind="ExternalOutput")

    ident_d = nc.inline_tensor(np.eye(128, dtype=np.float32), name="ident")
    ones16_d = nc.inline_tensor(np.ones((1, 128), np.float16), name="ones16")
    ones32_d = nc.inline_tensor(np.ones((1, 1), np.float32), name="ones32")

    with tile.TileContext(nc) as tc:
        with (
            tc.tile_pool(name="cst", bufs=1) as cst,
            tc.tile_pool(name="meta", bufs=1) as meta,
            tc.tile_pool(name="g", bufs=3) as gp,
            tc.tile_pool(name="x", bufs=3) as xp_pool,
            tc.tile_pool(name="s", bufs=4) as sp,
            tc.tile_pool(name="o", bufs=4) as op,
            tc.tile_pool(name="ps", bufs=2, space="PSUM") as pp,
            tc.tile_pool(name="ps2", bufs=2, space="PSUM") as pp2,
            tc.tile_pool(name="dram", bufs=1, space="DRAM") as dram,
        ):
            ident_t = cst.tile([128, 128], f32)
            nc.sync.dma_start(ident_t[:], ident_d[:])
            ones16 = cst.tile([1, 128], f16)
            nc.sync.dma_start(ones16[:], ones16_d[:])
            ones32 = cst.tile([1, 1], f32)
            nc.sync.dma_start(ones32[:], ones32_d[:])

            # weights resident
            W_t = {}
            w1t = cst.tile([128, D_H], f16, name="w1t")
            W_t[1] = [w1t]
            nc.gpsimd.dma_start(W_t[1][0][:], ext["W1"][:])
            for L in (2, 3):
                W_t[L] = []
                for j in range(3):
                    k = 128 if j < 2 else 48
                    w = cst.tile([128, D_H], f16, name=f"w{L}_{j}")
                    nc.gpsimd.dma_start(w[0:k, :], ext[f"W{L}"][j * 128:j * 128 + k, :])
                    W_t[L].append(w)
            b_t = {}
            for L in (1, 2, 3):
                b = cst.tile([1, D_H], f16, name=f"b{L}t")
                nc.gpsimd.dma_start(b[:], ext[f"b{L}"][:])
                b_t[L] = b
            fW1_t = []
            for j in range(3):
                k = 128 if j < 2 else 48
                w = cst.tile([128, 128], f32, name=f"fw1_{j}")
                nc.sync.dma_start(w[0:k, :], ext["fW1"][j * 128:j * 128 + k, :])
                fW1_t.append(w)
            fW2_t = cst.tile([128, 64], f32)
            nc.sync.dma_start(fW2_t[:], ext["fW2"][:])
            fW3_t = cst.tile([64, 1], f32)
            nc.sync.dma_start(fW3_t[:], ext["fW3"][:])
            fb_t = {}
            for nm, w in [("fb1", 128), ("fb2", 64), ("fb3", 1)]:
                b = cst.tile([1, w], f32, name=f"{nm}t")
                nc.sync.dma_start(b[:], ext[nm][:])
                fb_t[nm] = b

            # per-graph metadata in SBUF
            ix_t, rind_t, rout_t, rodf_t = {}, {}, {}, {}
            for g in range(3):
                Ck = g_meta[g][0]
                nch = sum(Ck)
                ix = meta.tile([128, nch * 8], i16, name=f"ix{g}")
                nc.sync.dma_start(ix[:], ext[f"ix{g}"][:])
                ix_t[g] = ix
                rind = meta.tile([128, NT], f32, name=f"rind{g}")
                tmp = meta.tile([128, NT], f32, tag="tmpd")
                nc.sync.dma_start(tmp[:], ext[f"ind{g}"][:])
                nc.scalar.sqrt(rind[:], tmp[:])
                nc.vector.reciprocal(rind[:], rind[:])
                rind_t[g] = rind
                rout = meta.tile([128, NT], f32, name=f"rout{g}")
                tmp2 = meta.tile([128, NT], f32, tag="tmpd2")
                nc.sync.dma_start(tmp2[:], ext[f"outd{g}"][:])
                nc.scalar.sqrt(rout[:], tmp2[:])
                nc.vector.reciprocal(rout[:], rout[:])
                rout_t[g] = rout
                rodf = meta.tile([128, 157], f32, name=f"rodf{g}")
                tmp3 = meta.tile([128, 157], f32, tag="tmpd3")
                nc.sync.dma_start(tmp3[:], ext[f"odf{g}"][:])
                nc.scalar.sqrt(rodf[:], tmp3[:])
                nc.vector.reciprocal(rodf[:], rodf[:])
                rodf_t[g] = rodf

            # DRAM feature tables
            xp = [dram.tile([N, D_IN], f16, name=f"xp{g}") for g in range(3)]
            hfA, hfB, shards = [], [], {}
            for g in range(3):
                hfA.append(dram.tile([N, DPAD], f16, addr_space="Shared", name=f"hfA{g}"))
                hfB.append(dram.tile([N, DPAD], f16, addr_space="Shared", name=f"hfB{g}"))
                shards[(g, 1)] = dram.tile([SH, DPAD], f16, name=f"sh1_{g}")
                shards[(g, 2)] = dram.tile([SH, DPAD], f16, name=f"sh2_{g}")
            pool_in = dram.tile([128, 3], f32)
            pool_out = dram.tile([128, 3], f32, addr_space="Shared")
            vec_b = dram.tile([1, 128], f32)

            macc = cst.tile([128, D_H], f32)
            nc.vector.memset(macc[:], 0.0)

            # ---- prescale: xp[g] = f16(x[g] * rsqrt(outdeg)), batched DMAs
            NB = 8  # tiles per DMA batch
            for g in range(3):
                with nc.named_scope(f"g{g}_prescale"):
                    rodf = rodf_t[g]
                    for b0 in range(0, 156, NB):
                        nb = min(NB, 156 - b0)
                        rows = nb * 128
                        r0 = b0 * 128
                        xt = xp_pool.tile([128, NB * 128], f32, tag="xt")
                        nc.sync.dma_start(
                            xt[:, 0:nb * 128].rearrange("p (c d) -> p c d", d=128),
                            ext[f"x{g}"][r0:r0 + rows, :].rearrange(
                                "(c p) d -> p c d", p=128))
                        xs = xp_pool.tile([128, NB * 128], f16, tag="xs")
                        for i in range(nb):
                            nc.scalar.activation(
                                xs[:, i * 128:(i + 1) * 128],
                                xt[:, i * 128:(i + 1) * 128], AF.Copy,
                                scale=rodf[:, b0 + i:b0 + i + 1])
                        nc.sync.dma_start(
                            xp[g][r0:r0 + rows, :].rearrange(
                                "(c p) d -> p c d", p=128),
                            xs[:, 0:nb * 128].rearrange("p (c d) -> p c d", d=128))
                    # tail tile 156: 32 rows
                    xt = xp_pool.tile([128, 128], f32, tag="xtt")
                    nc.sync.dma_start(xt[0:32, :], ext[f"x{g}"][19968:20000, :])
                    xs = xp_pool.tile([128, 128], f16, tag="xst")
                    nc.scalar.activation(xs[0:32, :], xt[0:32, :], AF.Copy,
                                         scale=rodf[0:32, 156:157])
                    nc.sync.dma_start(xp[g][19968:20000, :], xs[0:32, :])

            def layer(g, L):
                Ck = g_meta[g][0]
                src_tab = xp[g] if L == 1 else (hfA[g] if L == 2 else hfB[g])
                DL = D_IN if L == 1 else D_H
                DLP = D_IN if L == 1 else DPAD
                J = 1 if L == 1 else 3
                ix = ix_t[g]
                rind, rout = rind_t[g], rout_t[g]
                j0 = 0
                with nc.named_scope(f"g{g}L{L}"):
                    for t in range(NT):
                        rows = 128 if t < NT - 1 else SH - (NT - 1) * 128
                        ck = Ck[t]
                        nidx = ck * 128
                        gt = gp.tile([128, ck * DLP], f16, tag=f"g{L}")
                        psum = pp.tile([128, D_H], f32, tag="agg")
                        # one multi-packet gather per dst tile + host S stream
                        nc.gpsimd.dma_gather(
                            gt[:].rearrange("p (c e) -> p c e", e=DLP),
                            src_tab[:], ix[:, j0 * 8:(j0 + ck) * 8],
                            nidx, nidx, DLP, single_packet=False)
                        for c0 in range(0, ck, 8):
                            cn = min(8, ck - c0)
                            st = sp.tile([128, 8 * 128], f16, tag="st")
                            nc.sync.dma_start(
                                st[:, 0:cn * 128],
                                ext[f"S{g}"][:, (j0 + c0) * 128:(j0 + c0 + cn) * 128])
                            for c in range(cn):
                                cc = c0 + c
                                nc.tensor.matmul(psum[:, 0:DL],
                                                 st[:, c * 128:(c + 1) * 128],
                                                 gt[:, cc * DLP:cc * DLP + DL],
                                                 start=(cc == 0), stop=(cc == ck - 1))
                        j0 += ck
                        # scale by rsqrt(indeg), transpose, W matmul
                        zsb = op.tile([128, D_H], f32, tag="zsb")
                        nc.scalar.activation(zsb[:, 0:DL], psum[:, 0:DL], AF.Copy,
                                             scale=rind[:, t:t + 1])
                        psum2 = pp2.tile([128, D_H], f32, tag="wout")
                        for j in range(J):
                            k = 128 if (j < J - 1 or L == 1) else 48
                            tp = pp.tile([128, 128], f32, tag="tp")
                            nc.tensor.transpose(tp[0:k, :], zsb[:, j * 128:j * 128 + k],
                                                ident_t[:])
                            at = op.tile([128, 128], f16, tag="at")
                            nc.scalar.activation(at[0:k, :], tp[0:k, :], AF.Copy)
                            nc.tensor.matmul(psum2[:], at[0:k, :], W_t[L][j][0:k, :],
                                             start=(j == 0), stop=False)
                        nc.tensor.matmul(psum2[:], ones16[:], b_t[L][:],
                                         start=False, stop=True)
                        if L < 3:
                            hsb = op.tile([128, D_H], f16, tag="hsb")
                            nc.scalar.activation(hsb[:], psum2[:], AF.Relu,
                                                 scale=rout[:, t:t + 1])
                            nc.sync.dma_start(
                                shards[(g, L)][t * 128:t * 128 + rows, 0:D_H],
                                hsb[0:rows, :])
                        else:
                            hsb = op.tile([128, D_H], f32, tag="hsb3")
                            nc.scalar.activation(hsb[:], psum2[:], AF.Relu)
                            nc.vector.tensor_tensor(macc[0:rows, :], macc[0:rows, :],
                                                    hsb[0:rows, :], mybir.AluOpType.max)
                if L < 3:
                    dstf = hfA[g] if L == 1 else hfB[g]
                    with nc.named_scope(f"g{g}L{L}ag"):
                        nc.gpsimd.collective_compute(
                            "AllGather", mybir.AluOpType.bypass,
                            replica_groups=[core_ids],
                            ins=[shards[(g, L)].opt()],
                            outs=[dstf.opt()])

            for g in range(3):
                layer(g, 1)
            for g in range(3):
                layer(g, 2)
            for g in range(3):
                layer(g, 3)

            # max over partitions via transpose + reduce, AllReduce, MLP
            with nc.named_scope("tail"):
                pool_sb = cst.tile([128, 3], f32)
                nc.vector.memset(pool_sb[:], 0.0)
                for j in range(3):
                    k = 128 if j < 2 else 48
                    tp = pp.tile([128, 128], f32, tag="tp")
                    nc.tensor.transpose(tp[0:k, :], macc[:, j * 128:j * 128 + k],
                                        ident_t[:])
                    nc.vector.tensor_reduce(pool_sb[0:k, j:j + 1], tp[0:k, :],
                                            mybir.AxisListType.X, mybir.AluOpType.max)
                nc.sync.dma_start(pool_in[:], pool_sb[:])
                nc.gpsimd.collective_compute(
                    "AllReduce", mybir.AluOpType.max, replica_groups=[core_ids],
                    ins=[pool_in.opt()], outs=[pool_out.opt()])
                pool_t = cst.tile([128, 3], f32)
                nc.sync.dma_start(pool_t[:], pool_out[:])

                z1p = pp2.tile([1, 128], f32, tag="z")
                for j in range(3):
                    k = 128 if j < 2 else 48
                    nc.tensor.matmul(z1p[:], pool_t[0:k, j:j + 1], fW1_t[j][0:k, :],
                                     start=(j == 0), stop=False)
                nc.tensor.matmul(z1p[:], ones32[:], fb_t["fb1"][:], start=False, stop=True)
                z1s = cst.tile([1, 128], f32)
                nc.scalar.activation(z1s[:], z1p[:], AF.Relu)
                nc.sync.dma_start(vec_b[:], z1s[:])
                z1T = cst.tile([128, 1], f32)
                nc.sync.dma_start(z1T[:], vec_b[0, :].rearrange("(p o) -> p o", o=1))
                z2p = pp2.tile([1, 64], f32, tag="z")
                nc.tensor.matmul(z2p[:], z1T[:], fW2_t[:], start=True, stop=False)
                nc.tensor.matmul(z2p[:], ones32[:], fb_t["fb2"][:], start=False, stop=True)
                z2s = cst.tile([1, 64], f32)
                nc.scalar.activation(z2s[:], z2p[:], AF.Relu)
                nc.sync.dma_start(vec_b[0:1, 0:64], z2s[:])
                z2T = cst.tile([64, 1], f32)
                nc.sync.dma_start(z2T[:], vec_b[0, 0:64].rearrange("(p o) -> p o", o=1))
                z3p = pp2.tile([1, 1], f32, tag="z")
                nc.tensor.matmul(z3p[:], z2T[:], fW3_t[:], start=True, stop=False)
                nc.tensor.matmul(z3p[:], ones32[:], fb_t["fb3"][:], start=False, stop=True)
                ys = cst.tile([1, 1], f32)
                nc.scalar.activation(ys[:], z3p[:], AF.Sigmoid)
                nc.sync.dma_start(y_ext[:], ys[:])

    nc.compile()
    return nc


def kernel(**inputs):
    g_meta = []
    for g, (s, d) in enumerate([("src1", "dst1"), ("src2", "dst2"), ("src3", "dst3")]):
        g_meta.append(_prep_graph(inputs[s], inputs[d]))
    nc = _build(g_meta)
    in_maps = []
    for c in range(NC_):
        m = {}
        for g, xn in enumerate(["x1", "x2", "x3"]):
            Ck, idxw, smat, ind, outd, odf = g_meta[g]
            m[f"x{g}"] = np.asarray(inputs[xn], np.float32)
            m[f"ix{g}"] = idxw[c]
            m[f"S{g}"] = smat[c]
            m[f"ind{g}"] = ind[c]
            m[f"outd{g}"] = outd[c]
            m[f"odf{g}"] = odf
        for nm in ["W1", "W2", "W3", "fW2"]:
            m[nm] = np.asarray(inputs[nm], np.float32)
        m["fW1"] = np.asarray(inputs["fW1"], np.float32)
        m["fW3"] = np.asarray(inputs["fW3"], np.float32).reshape(64, 1)
        for nm in ["b1", "b2", "b3", "fb1", "fb2", "fb3"]:
            m[nm] = np.asarray(inputs[nm], np.float32).reshape(1, -1)
        in_maps.append(m)
    trace = bool(int(os.environ.get("KTRACE", "0")))
    tmpdir = os.environ.get("KTRACE_DIR") or None
    res = run_bass_kernel_spmd(nc, in_maps, core_ids, trace=trace, tmpdir=tmpdir)
    kernel.last_results = res
    return np.asarray(res.results[0]["y"], np.float32).reshape(1)


# revision 30
# speedup vs baseline: 1.9197x; 1.3368x over previous
"""3-branch GCN (DGL GraphConv x3 + max-pool + MLP head) on 8 TRN2 NeuronCores.

Sharding: destination nodes (2500/core). Per layer, each core batch-gathers all
src rows for one 128-dst tile with a single dma_gather (custom SWDGE gather:
~1us fixed + 0.34ns/row), aggregates via one-hot fp16 matmuls into PSUM,
applies the dense W matmul per dst tile, and AllGathers the layer output shards
for the next layer. The three graphs are emitted interleaved so each AllGather
overlaps the other graphs' compute. Max-pool is local + AllReduce(max); the
tiny MLP head runs replicated.
"""
import os
import numpy as np
import concourse.bass as bass
import concourse.bacc as bacc
import concourse.tile as tile
import concourse.mybir as mybir
from concourse.bass_utils import run_bass_kernel_spmd

NC_ = 8
N = 20000
E = 320000
SH = N // NC_          # 2500 nodes per core
NT = 20                # dst tiles per core (19 full + 68-node partial)
D_IN, D_H = 128, 304
DPAD = 384             # fp16 row pad -> 768B rows (256B multiple for dma_gather)
f16, f32 = mybir.dt.float16, mybir.dt.float32
i32, i16 = mybir.dt.int32, mybir.dt.int16
AF = mybir.ActivationFunctionType
core_ids = list(range(NC_))


def _prep_graph(src, dst):
    """Per-core chunked edge metadata with core-uniform chunk counts."""
    src = np.asarray(src).astype(np.int64)
    dst = np.asarray(dst).astype(np.int64)
    outdeg = np.bincount(src, minlength=N).clip(1).astype(np.float32)
    indeg = np.bincount(dst, minlength=N).clip(1).astype(np.float32)
    per_core = []
    for c in range(NC_):
        m = (dst // SH) == c
        es, ed = src[m], dst[m] - c * SH
        tiles = []
        for t in range(NT):
            tm = (ed // 128) == t
            tiles.append((es[tm], ed[tm] - t * 128))
        per_core.append(tiles)
    # uniform chunk count per tile slot
    Ck = [max(int(np.ceil(len(per_core[c][t][0]) / 128)) for c in range(NC_)) or 1
          for t in range(NT)]
    nchunks = sum(Ck)
    # wrapped int16 gather indices: edge i of a tile -> [i % 16, i // 16]
    idxw = np.zeros((NC_, 128, nchunks * 8), np.int16)
    drel = np.full((NC_, 128, nchunks), -1.0, np.float32)  # -1 -> zero S col
    for c in range(NC_):
        j0 = 0
        for t in range(NT):
            es, er = per_core[c][t]
            npad = Ck[t] * 128
            e_s = np.full(npad, 0, np.int64)      # pad -> row 0 (weight 0)
            e_r = np.full(npad, -1.0, np.float32)
            e_s[:len(es)] = es
            e_r[:len(er)] = er
            drel[c, :, j0:j0 + Ck[t]] = e_r.reshape(Ck[t], 128).T
            # wrapped [16, cols] block replicated to all 8 Q7 core groups
            idxw[c, :, j0 * 8:(j0 + Ck[t]) * 8] = np.tile(
                e_s.astype(np.int16).reshape(Ck[t] * 8, 16).T, (8, 1))
            j0 += Ck[t]
    # per-core slot-ordered degree arrays [128, NT]
    ind = np.ones((NC_, 128, NT), np.float32)
    outd = np.ones((NC_, 128, NT), np.float32)
    for c in range(NC_):
        for t in range(NT):
            lo = c * SH + t * 128
            hi = min(lo + 128, (c + 1) * SH)
            ind[c, :hi - lo, t] = indeg[lo:hi]
            outd[c, :hi - lo, t] = outdeg[lo:hi]
    # full outdeg, node n -> [n % 128, n // 128] (lane-major tiles)
    tmp = np.ones(157 * 128, np.float32)
    tmp[:N] = outdeg
    odf = tmp.reshape(157, 128).T.copy()
    # host-precomputed one-hot S matrices: smat[:, j*128+m] = (drel[:, j] == m)
    smat = (drel[:, :, :, None] ==
            np.arange(128, dtype=np.float32)[None, None, None, :]
            ).astype(np.float16).reshape(NC_, 128, nchunks * 128)
    return Ck, idxw, smat, ind, outd, odf


def _build(g_meta):
    nc = bacc.Bacc(None, target_bir_lowering=False, num_swdge_queues=2)
    ext = {}
    for g in range(3):
        Ck, idxw, drel, ind, outd, odf = g_meta[g]
        nch = sum(Ck)
        ext[f"x{g}"] = nc.dram_tensor(f"x{g}", [N, D_IN], f32, kind="ExternalInput")
        ext[f"ix{g}"] = nc.dram_tensor(f"ix{g}", [128, nch * 8], i16, kind="ExternalInput")
        ext[f"S{g}"] = nc.dram_tensor(f"S{g}", [128, nch * 128], f16, kind="ExternalInput")
        ext[f"ind{g}"] = nc.dram_tensor(f"ind{g}", [128, NT], f32, kind="ExternalInput")
        ext[f"outd{g}"] = nc.dram_tensor(f"outd{g}", [128, NT], f32, kind="ExternalInput")
        ext[f"odf{g}"] = nc.dram_tensor(f"odf{g}", [128, 157], f32, kind="ExternalInput")
    for nm, shp in [("W1", [D_IN, D_H]), ("W2", [D_H, D_H]), ("W3", [D_H, D_H]),
                    ("b1", [1, D_H]), ("b2", [1, D_H]), ("b3", [1, D_H]),
                    ("fW1", [D_H, 128]), ("fb1", [1, 128]), ("fW2", [128, 64]),
                    ("fb2", [1, 64]), ("fW3", [64, 1]), ("fb3", [1, 1])]:
        ext[nm] = nc.dram_tensor(nm, shp, f32, kind="ExternalInput")
    y_ext = nc.dram_tensor("y", [1, 1], f32, kind="ExternalOutput")

    ident_d = nc.inline_tensor(np.eye(128, dtype=np.float32), name="ident")
    ones16_d = nc.inline_tensor(np.ones((1, 128), np.float16), name="ones16")
    ones32_d = nc.inline_tensor(np.ones((1, 1), np.float32), name="ones32")

    with tile.TileContext(nc) as tc:
        with (
            tc.tile_pool(name="cst", bufs=1) as cst,
            tc.tile_pool(name="meta", bufs=1) as meta,
            tc.tile_pool(name="g", bufs=3) as gp,
            tc.tile_pool(name="x", bufs=3) as xp_pool,
            tc.tile_pool(name="s", bufs=4) as sp,
            tc.tile_pool(name="o", bufs=4) as op,
            tc.tile_pool(name="ps", bufs=2, space="PSUM") as pp,
            tc.tile_pool(name="ps2", bufs=2, space="PSUM") as pp2,
            tc.tile_pool(name="dram", bufs=1, space="DRAM") as dram,
        ):
            ident_t = cst.tile([128, 128], f32)
            nc.sync.dma_start(ident_t[:], ident_d[:])
            ones16 = cst.tile([1, 128], f16)
            nc.sync.dma_start(ones16[:], ones16_d[:])
            ones32 = cst.tile([1, 1], f32)
            nc.sync.dma_start(ones32[:], ones32_d[:])

            # weights resident
            W_t = {}
            w1t = cst.tile([128, D_H], f16, name="w1t")
            W_t[1] = [w1t]
            nc.gpsimd.dma_start(W_t[1][0][:], ext["W1"][:])
            for L in (2, 3):
                W_t[L] = []
                for j in range(3):
                    k = 128 if j < 2 else 48
                    w = cst.tile([128, D_H], f16, name=f"w{L}_{j}")
                    nc.gpsimd.dma_start(w[0:k, :], ext[f"W{L}"][j * 128:j * 128 + k, :])
                    W_t[L].append(w)
            b_t = {}
            for L in (1, 2, 3):
                b = cst.tile([1, D_H], f16, name=f"b{L}t")
                nc.gpsimd.dma_start(b[:], ext[f"b{L}"][:])
                b_t[L] = b
            fW1_t = []
            for j in range(3):
                k = 128 if j < 2 else 48
                w = cst.tile([128, 128], f32, name=f"fw1_{j}")
                nc.sync.dma_start(w[0:k, :], ext["fW1"][j * 128:j * 128 + k, :])
                fW1_t.append(w)
            fW2_t = cst.tile([128, 64], f32)
            nc.sync.dma_start(fW2_t[:], ext["fW2"][:])
            fW3_t = cst.tile([64, 1], f32)
            nc.sync.dma_start(fW3_t[:], ext["fW3"][:])
            fb_t = {}
            for nm, w in [("fb1", 128), ("fb2", 64), ("fb3", 1)]:
                b = cst.tile([1, w], f32, name=f"{nm}t")
                nc.sync.dma_start(b[:], ext[nm][:])
                fb_t[nm] = b

            # per-graph metadata in SBUF
            ix_t, rind_t, rout_t, rodf_t = {}, {}, {}, {}
            for g in range(3):
                Ck = g_meta[g][0]
                nch = sum(Ck)
                ix = meta.tile([128, nch * 8], i16, name=f"ix{g}")
                nc.sync.dma_start(ix[:], ext[f"ix{g}"][:])
                ix_t[g] = ix
                rind = meta.tile([128, NT], f32, name=f"rind{g}")
                tmp = meta.tile([128, NT], f32, tag="tmpd")
                nc.sync.dma_start(tmp[:], ext[f"ind{g}"][:])
                nc.scalar.sqrt(rind[:], tmp[:])
                nc.vector.reciprocal(rind[:], rind[:])
                rind_t[g] = rind
                rout = meta.tile([128, NT], f32, name=f"rout{g}")
                tmp2 = meta.tile([128, NT], f32, tag="tmpd2")
                nc.sync.dma_start(tmp2[:], ext[f"outd{g}"][:])
                nc.scalar.sqrt(rout[:], tmp2[:])
                nc.vector.reciprocal(rout[:], rout[:])
                rout_t[g] = rout
                rodf = meta.tile([128, 157], f32, name=f"rodf{g}")
                tmp3 = meta.tile([128, 157], f32, tag="tmpd3")
                nc.sync.dma_start(tmp3[:], ext[f"odf{g}"][:])
                nc.scalar.sqrt(rodf[:], tmp3[:])
                nc.vector.reciprocal(rodf[:], rodf[:])
                rodf_t[g] = rodf

            # DRAM feature tables
            xp = [dram.tile([N, D_IN], f16, name=f"xp{g}") for g in range(3)]
            hfA, hfB, shards = [], [], {}
            for g in range(3):
                hfA.append(dram.tile([N, DPAD], f16, addr_space="Shared", name=f"hfA{g}"))
                hfB.append(dram.tile([N, DPAD], f16, addr_space="Shared", name=f"hfB{g}"))
                shards[(g, 1)] = dram.tile([SH, DPAD], f16, name=f"sh1_{g}")
                shards[(g, 2)] = dram.tile([SH, DPAD], f16, name=f"sh2_{g}")
            pool_in = dram.tile([128, 3], f32)
            pool_out = dram.tile([128, 3], f32, addr_space="Shared")
            vec_b = dram.tile([1, 128], f32)

            macc = cst.tile([128, D_H], f32)
            nc.vector.memset(macc[:], 0.0)

            # ---- prescale: xp[g] = f16(x[g] * rsqrt(outdeg)), batched DMAs
            NB = 8  # tiles per DMA batch
            for g in range(3):
                with nc.named_scope(f"g{g}_prescale"):
                    rodf = rodf_t[g]
                    for b0 in range(0, 156, NB):
                        nb = min(NB, 156 - b0)
                        rows = nb * 128
                        r0 = b0 * 128
                        xt = xp_pool.tile([128, NB * 128], f32, tag="xt")
                        nc.sync.dma_start(
                            xt[:, 0:nb * 128].rearrange("p (c d) -> p c d", d=128),
                            ext[f"x{g}"][r0:r0 + rows, :].rearrange(
                                "(c p) d -> p c d", p=128))
                        xs = xp_pool.tile([128, NB * 128], f16, tag="xs")
                        for i in range(nb):
                            nc.scalar.activation(
                                xs[:, i * 128:(i + 1) * 128],
                                xt[:, i * 128:(i + 1) * 128], AF.Copy,
                                scale=rodf[:, b0 + i:b0 + i + 1])
                        nc.sync.dma_start(
                            xp[g][r0:r0 + rows, :].rearrange(
                                "(c p) d -> p c d", p=128),
                            xs[:, 0:nb * 128].rearrange("p (c d) -> p c d", d=128))
                    # tail tile 156: 32 rows
                    xt = xp_pool.tile([128, 128], f32, tag="xtt")
                    nc.sync.dma_start(xt[0:32, :], ext[f"x{g}"][19968:20000, :])
                    xs = xp_pool.tile([128, 128], f16, tag="xst")
                    nc.scalar.activation(xs[0:32, :], xt[0:32, :], AF.Copy,
                                         scale=rodf[0:32, 156:157])
                    nc.sync.dma_start(xp[g][19968:20000, :], xs[0:32, :])

            def layer(g, L):
                Ck = g_meta[g][0]
                src_tab = xp[g] if L == 1 else (hfA[g] if L == 2 else hfB[g])
                DL = D_IN if L == 1 else D_H
                DLP = D_IN if L == 1 else DPAD
                J = 1 if L == 1 else 3
                ix = ix_t[g]
                rind, rout = rind_t[g], rout_t[g]
                j0 = 0
                with nc.named_scope(f"g{g}L{L}"):
                    for t in range(NT):
                        rows = 128 if t < NT - 1 else SH - (NT - 1) * 128
                        ck = Ck[t]
                        nidx = ck * 128
                        gt = gp.tile([128, ck * DLP], f16, tag=f"g{L}")
                        psum = pp.tile([128, D_H], f32, tag="agg")
                        # one multi-packet gather per dst tile + host S stream
                        nc.gpsimd.dma_gather(
                            gt[:].rearrange("p (c e) -> p c e", e=DLP),
                            src_tab[:], ix[:, j0 * 8:(j0 + ck) * 8],
                            nidx, nidx, DLP, single_packet=False,
                            queue_num=t % 2)
                        for c0 in range(0, ck, 8):
                            cn = min(8, ck - c0)
                            st = sp.tile([128, 8 * 128], f16, tag="st")
                            nc.sync.dma_start(
                                st[:, 0:cn * 128],
                                ext[f"S{g}"][:, (j0 + c0) * 128:(j0 + c0 + cn) * 128])
                            for c in range(cn):
                                cc = c0 + c
                                nc.tensor.matmul(psum[:, 0:DL],
                                                 st[:, c * 128:(c + 1) * 128],
                                                 gt[:, cc * DLP:cc * DLP + DL],
                                                 start=(cc == 0), stop=(cc == ck - 1))
                        j0 += ck
                        # scale by rsqrt(indeg), transpose, W matmul
                        zsb = op.tile([128, D_H], f32, tag="zsb")
                        nc.scalar.activation(zsb[:, 0:DL], psum[:, 0:DL], AF.Copy,
                                             scale=rind[:, t:t + 1])
                        psum2 = pp2.tile([128, D_H], f32, tag="wout")
                        for j in range(J):
                            k = 128 if (j < J - 1 or L == 1) else 48
                            tp = pp.tile([128, 128], f32, tag="tp")
                            nc.tensor.transpose(tp[0:k, :], zsb[:, j * 128:j * 128 + k],
                                                ident_t[:])
                            at = op.tile([128, 128], f16, tag="at")
                            nc.scalar.activation(at[0:k, :], tp[0:k, :], AF.Copy)
                            nc.tensor.matmul(psum2[:], at[0:k, :], W_t[L][j][0:k, :],
                                             start=(j == 0), stop=False)
                        nc.tensor.matmul(psum2[:], ones16[:], b_t[L][:],
                                         start=False, stop=True)
                        if L < 3:
                            hsb = op.tile([128, D_H], f16, tag="hsb")
                            nc.scalar.activation(hsb[:], psum2[:], AF.Relu,
                                                 scale=rout[:, t:t + 1])
                            nc.sync.dma_start(
                                shards[(g, L)][t * 128:t * 128 + rows, 0:D_H],
                                hsb[0:rows, :])
                        else:
                            hsb = op.tile([128, D_H], f32, tag="hsb3")
                            nc.scalar.activation(hsb[:], psum2[:], AF.Relu)
                            nc.vector.tensor_tensor(macc[0:rows, :], macc[0:rows, :],
                                                    hsb[0:rows, :], mybir.AluOpType.max)
                if L < 3:
                    dstf = hfA[g] if L == 1 else hfB[g]
                    with nc.named_scope(f"g{g}L{L}ag"):
                        nc.gpsimd.collective_compute(
                            "AllGather", mybir.AluOpType.bypass,
                            replica_groups=[core_ids],
                            ins=[shards[(g, L)].opt()],
                            outs=[dstf.opt()])

            for g in range(3):
                layer(g, 1)
            for g in range(3):
                layer(g, 2)
            for g in range(3):
                layer(g, 3)

            # max over partitions via transpose + reduce, AllReduce, MLP
            with nc.named_scope("tail"):
                pool_sb = cst.tile([128, 3], f32)
                nc.vector.memset(pool_sb[:], 0.0)
                for j in range(3):
                    k = 128 if j < 2 else 48
                    tp = pp.tile([128, 128], f32, tag="tp")
                    nc.tensor.transpose(tp[0:k, :], macc[:, j * 128:j * 128 + k],
                                        ident_t[:])
                    nc.vector.tensor_reduce(pool_sb[0:k, j:j + 1], tp[0:k, :],
                                            mybir.AxisListType.X, mybir.AluOpType.max)
                nc.sync.dma_start(pool_in[:], pool_sb[:])
                nc.gpsimd.collective_compute(
                    "AllReduce", mybir.AluOpType.max, replica_groups=[core_ids],
                    ins=[pool_in.opt()], outs=[pool_out.opt()])
                pool_t = cst.tile([128, 3], f32)
                nc.sync.dma_start(pool_t[:], pool_out[:])

                z1p = pp2.tile([1, 128], f32, tag="z")
                for j in range(3):
                    k = 128 if j < 2 else 48
                    nc.tensor.matmul(z1p[:], pool_t[0:k, j:j + 1], fW1_t[j][0:k, :],
                                     start=(j == 0), stop=False)
                nc.tensor.matmul(z1p[:], ones32[:], fb_t["fb1"][:], start=False, stop=True)
                z1s = cst.tile([1, 128], f32)
                nc.scalar.activation(z1s[:], z1p[:], AF.Relu)
                nc.sync.dma_start(vec_b[:], z1s[:])
                z1T = cst.tile([128, 1], f32)
                nc.sync.dma_start(z1T[:], vec_b[0, :].rearrange("(p o) -> p o", o=1))
                z2p = pp2.tile([1, 64], f32, tag="z")
                nc.tensor.matmul(z2p[:], z1T[:], fW2_t[:], start=True, stop=False)
                nc.tensor.matmul(z2p[:], ones32[:], fb_t["fb2"][:], start=False, stop=True)
                z2s = cst.tile([1, 64], f32)
                nc.scalar.activation(z2s[:], z2p[:], AF.Relu)
                nc.sync.dma_start(vec_b[0:1, 0:64], z2s[:])
                z2T = cst.tile([64, 1], f32)
                nc.sync.dma_start(z2T[:], vec_b[0, 0:64].rearrange("(p o) -> p o", o=1))
                z3p = pp2.tile([1, 1], f32, tag="z")
                nc.tensor.matmul(z3p[:], z2T[:], fW3_t[:], start=True, stop=False)
                nc.tensor.matmul(z3p[:], ones32[:], fb_t["fb3"][:], start=False, stop=True)
                ys = cst.tile([1, 1], f32)
                nc.scalar.activation(ys[:], z3p[:], AF.Sigmoid)
                nc.sync.dma_start(y_ext[:], ys[:])

    nc.compile()
    return nc


def kernel(**inputs):
    g_meta = []
    for g, (s, d) in enumerate([("src1", "dst1"), ("src2", "dst2"), ("src3", "dst3")]):
        g_meta.append(_prep_graph(inputs[s], inputs[d]))
    nc = _build(g_meta)
    in_maps = []
    for c in range(NC_):
        m = {}
        for g, xn in enumerate(["x1", "x2", "x3"]):
            Ck, idxw, smat, ind, outd, odf = g_meta[g]
            m[f"x{g}"] = np.asarray(inputs[xn], np.float32)
            m[f"ix{g}"] = idxw[c]
            m[f"S{g}"] = smat[c]
            m[f"ind{g}"] = ind[c]
            m[f"outd{g}"] = outd[c]
            m[f"odf{g}"] = odf
        for nm in ["W1", "W2", "W3", "fW2"]:
            m[nm] = np.asarray(inputs[nm], np.float32)
        m["fW1"] = np.asarray(inputs["fW1"], np.float32)
        m["fW3"] = np.asarray(inputs["fW3"], np.float32).reshape(64, 1)
        for nm in ["b1", "b2", "b3", "fb1", "fb2", "fb3"]:
            m[nm] = np.asarray(inputs[nm], np.float32).reshape(1, -1)
        in_maps.append(m)
    trace = bool(int(os.environ.get("KTRACE", "0")))
    tmpdir = os.environ.get("KTRACE_DIR") or None
    res = run_bass_kernel_spmd(nc, in_maps, core_ids, trace=trace, tmpdir=tmpdir)
    kernel.last_results = res
    return np.asarray(res.results[0]["y"], np.float32).reshape(1)
